# revision 1
# baseline (speedup 1.0000x reference)
"""Bass/Tile TRN2 kernel for nn_DimensionScaledEqProp.

Data-parallel over batch: x rows sharded across 8 NeuronCores, weights
replicated. Per-core state (h) stays resident in SBUF across the 30
sequential steps. fp16 matmul operands, fp32 accumulation/state.

Self-contained: hardcodes shapes; host side does sharding, spectral-norm
sigma (tiny: 60 matvecs), weight folding/transposition, and the final
gather/transpose.
"""
import sys
import numpy as np

for _p in ("/opt/trn_rl_repo", "/root/.axon_site/_ro/trn_rl_repo"):
    if _p not in sys.path:
        sys.path.append(_p)

B, DIN, DH, DOUT = 4096, 512, 1024, 256
DFF = 4 * DH
STEPS = 30
N_CORES = 8
R = B // N_CORES  # rows per core = 512
GAMMA = 0.5 * min(1.0, float(np.sqrt(64.0 / DIN)))
LN_EPS = 1e-5

KD = DH // 128    # 8  k-tiles over DH
FD = DFF // 128   # 32 f-tiles over DFF
RD = R // 128     # 4  row-tiles per core
ID = DIN // 128   # 4  k-tiles over DIN
OD = DOUT // 128  # 2  out-tiles over DOUT
NH = DH // 512    # 2  psum halves over DH

_CACHE = {}


def _build_program(steps: int):
    import concourse.bass as bass
    import concourse.bacc as bacc
    import concourse.mybir as mybir
    from concourse import tile, masks

    f16 = mybir.dt.float16
    f32 = mybir.dt.float32
    AF = mybir.ActivationFunctionType
    OP = mybir.AluOpType

    nc = bacc.Bacc("TRN2", target_bir_lowering=False, debug=False,
                   enable_asserts=True, num_devices=N_CORES)

    xT_d = nc.dram_tensor("xT", [DIN, R], f16, kind="ExternalInput")
    ewT_d = nc.dram_tensor("ewT", [DIN, DH], f16, kind="ExternalInput")
    w1t_d = nc.dram_tensor("w1t", [DH, DFF], f16, kind="ExternalInput")
    b1e_d = nc.dram_tensor("b1e", [DFF, 1], f32, kind="ExternalInput")
    w2t_d = nc.dram_tensor("w2t", [DFF, DH], f16, kind="ExternalInput")
    vb1_d = nc.dram_tensor("vb1", [1, DH], f32, kind="ExternalInput")
    vb2_d = nc.dram_tensor("vb2", [1, DH], f32, kind="ExternalInput")
    hwt_d = nc.dram_tensor("hwt", [DH, DOUT], f16, kind="ExternalInput")
    hb_d = nc.dram_tensor("hb", [DOUT, 1], f32, kind="ExternalInput")
    outT_d = nc.dram_tensor("outT", [DOUT, R], f32, kind="ExternalOutput")

    with tile.TileContext(nc) as tc:
        with (
            tc.tile_pool(name="wp", bufs=1) as wp,
            tc.tile_pool(name="sp", bufs=1) as sp,
            tc.tile_pool(name="wk", bufs=2) as wk,
            tc.tile_pool(name="stp", bufs=2) as stp,
            tc.tile_pool(name="pst", bufs=4, space="PSUM") as pst,
            tc.tile_pool(name="ps1", bufs=2, space="PSUM") as ps1p,
            tc.tile_pool(name="ps2", bufs=2, space="PSUM") as ps2p,
        ):
            # ---- persistent weights / constants ----
            w1 = [wp.tile([128, DFF], f16, name=f"w1_{k}") for k in range(KD)]
            w2 = [wp.tile([128, DH], f16, name=f"w2_{f}") for f in range(FD)]
            hwt = [wp.tile([128, DOUT], f16, name=f"hwt_{k}") for k in range(KD)]
            b1s = wp.tile([128, FD], f32, name="b1s")
            hbs = wp.tile([128, OD], f32, name="hbs")
            ident = wp.tile([128, 128], f16, name="ident")

            # ---- persistent state ----
            h = [sp.tile([128, DH], f32, name=f"h_{r}") for r in range(RD)]
            xeg = [sp.tile([128, DH], f16, name=f"xeg_{r}") for r in range(RD)]
            hnT = [sp.tile([128, R], f16, name=f"hnT_{k}") for k in range(KD)]

            masks.make_identity(nc, ident[:])

            # ---- embed (transient pool, released before the step loop) ----
            with tc.tile_pool(name="ep", bufs=1) as ep:
                xts = [ep.tile([128, R], f16, name=f"xts_{i}")
                       for i in range(ID)]
                ewt = [ep.tile([128, DH], f16, name=f"ewt_{i}")
                       for i in range(ID)]
                bc1 = ep.tile([128, DH], f32, name="bc1")
                bc2 = ep.tile([128, DH], f32, name="bc2")
                for i in range(ID):
                    nc.sync.dma_start(
                        xts[i][:], xT_d.ap()[i * 128:(i + 1) * 128, :])
                    nc.sync.dma_start(
                        ewt[i][:], ewT_d.ap()[i * 128:(i + 1) * 128, :])
                nc.sync.dma_start(bc1[0:1, :], vb1_d.ap())
                nc.sync.dma_start(bc2[0:1, :], vb2_d.ap())
                nc.gpsimd.partition_broadcast(bc1[:], bc1[0:1, :])
                nc.gpsimd.partition_broadcast(bc2[:], bc2[0:1, :])

                # weight loads AFTER embed inputs: embed matmuls start
                # immediately; w1/w2 stream in behind them
                for k in range(KD):
                    nc.sync.dma_start(
                        w1[k][:], w1t_d.ap()[k * 128:(k + 1) * 128, :])
                nc.sync.dma_start(
                    b1s[:], b1e_d.ap().rearrange("(f p) o -> p (f o)", p=128))
                nc.sync.dma_start(
                    hbs[:], hb_d.ap().rearrange("(t p) o -> p (t o)", p=128))

                # h0 = x @ ewT + embed_b ; xeg = g*h0 + g*b2 (f16)
                for r in range(RD):
                    for half in range(NH):
                        sl = slice(half * 512, (half + 1) * 512)
                        pe = ps1p.tile([128, 512], f32, tag="ps1", name="pe")
                        for i in range(ID):
                            nc.tensor.matmul(
                                pe[:], xts[i][:, r * 128:(r + 1) * 128],
                                ewt[i][:, sl],
                                start=(i == 0), stop=(i == ID - 1))
                        nc.vector.tensor_tensor(
                            h[r][:, sl], pe[:], bc1[:, sl], op=OP.add)
                        nc.vector.scalar_tensor_tensor(
                            xeg[r][:, sl], h[r][:, sl], GAMMA, bc2[:, sl],
                            op0=OP.mult, op1=OP.add)

            # ---- initial LN stats on h0 (ACT sqrt once; hides table load) ----
            mv0 = stp.tile([128, RD * 2], f32, tag="mv", name="mv_init")
            for r in range(RD):
                st6 = stp.tile([128, 12], f32, tag="st6", name=f"st6_i_{r}")
                for c in range(2):
                    nc.vector.bn_stats(
                        st6[:, c * 6:(c + 1) * 6],
                        h[r][:, c * 512:(c + 1) * 512])
                nc.vector.bn_aggr(
                    mv0[:].rearrange("p (r x) -> p r x", x=2)[:, r], st6[:])
            mvv0 = mv0[:].rearrange("p (r x) -> p r x", x=2)
            ve0 = stp.tile([128, RD], f32, tag="ve", name="ve_init")
            nc.vector.tensor_scalar(
                ve0[:], mv0[:].rearrange("p (r x) -> p x r", x=2)[:, 1], LN_EPS, None, op0=OP.add)
            rv0 = stp.tile([128, RD], f32, tag="rv", name="rv_init")
            nc.vector.reciprocal(rv0[:], ve0[:])
            rs_prev = stp.tile([128, RD], f32, tag="rs", name="rs_init")
            nc.scalar.activation(rs_prev[:], rv0[:], AF.Sqrt)

            # ---- hidT pool reuses the embed pool space ----
            with tc.tile_pool(name="hp", bufs=1) as hp:
                hidT = [hp.tile([128, R], f16, name=f"hidT_{f}")
                        for f in range(FD)]

                def rstd_newton(y_out, y_seed, var_ap, tag_sfx, n_iter=2):
                    """y_out[128,1] = 1/sqrt(var+eps) via Newton from seed."""
                    hv = stp.tile([128, 1], f32, tag="hv",
                                  name=f"hv_{tag_sfx}")
                    nc.vector.tensor_scalar(
                        hv[:], var_ap, -0.5, -0.5 * LN_EPS,
                        op0=OP.mult, op1=OP.add)
                    y = y_seed
                    for it in range(n_iter):
                        a = stp.tile([128, 1], f32, tag="nwa",
                                     name=f"nwa_{tag_sfx}_{it}")
                        nc.vector.tensor_tensor(a[:], y, y, op=OP.mult)
                        nc.vector.tensor_scalar(
                            a[:], a[:], hv[:], 1.5, op0=OP.mult, op1=OP.add)
                        if it == n_iter - 1:
                            nc.vector.tensor_tensor(y_out, y, a[:], op=OP.mult)
                        else:
                            yn = stp.tile([128, 1], f32, tag="nwy",
                                          name=f"nwy_{tag_sfx}_{it}")
                            nc.vector.tensor_tensor(yn[:], y, a[:], op=OP.mult)
                            y = yn[:]

                # normalize h0 -> hnT for step 0 (one Newton polish on seed)
                mv_p, rs_p = mv0, rs_prev
                rs_fix = stp.tile([128, RD], f32, tag="rsf", name="rs_fix")
                for r in range(RD):
                    rstd_newton(rs_fix[:, r:r + 1], rs_prev[:, r:r + 1],
                                mv0[:, 2 * r + 1:2 * r + 2], f"i{r}", n_iter=1)
                rs_p = rs_fix

                def normalize(r, mean_ap, rs_col, sfx):
                    nmu = stp.tile([128, 1], f32, tag="nmu", name=f"nmu_{sfx}")
                    nc.vector.scalar_tensor_tensor(
                        nmu[:], mean_ap, -1.0, rs_col,
                        op0=OP.mult, op1=OP.mult)
                    hn16 = wk.tile([128, DH], f16, tag=f"hn16_{r}",
                                   name=f"hn16_{sfx}", bufs=1)
                    # two half-width ops so transposes of the low half can
                    # start before the high half is normalized
                    for half in range(NH):
                        sl = slice(half * 512, (half + 1) * 512)
                        nc.vector.tensor_scalar(
                            hn16[:, sl], h[r][:, sl], rs_col, nmu[:],
                            op0=OP.mult, op1=OP.add)
                    return hn16

                def transposes(r, hn16, sfx):
                    for k in range(KD):
                        tp = pst.tile([128, 128], f16, tag="tp",
                                      name=f"tp_{sfx}_{k}")
                        nc.tensor.transpose(
                            tp[:], hn16[:, k * 128:(k + 1) * 128], ident[:])
                        if k % 2 == 0:
                            nc.vector.tensor_copy(
                                hnT[k][:, r * 128:(r + 1) * 128], tp[:])
                        else:
                            nc.scalar.copy(
                                hnT[k][:, r * 128:(r + 1) * 128], tp[:])

                for r in range(RD):
                    hn = normalize(r, mv0[:, 2 * r:2 * r + 1],
                                   rs_p[:, r:r + 1], f"s0_{r}")
                    transposes(r, hn, f"s0_{r}")

                for s in range(steps):
                    last = (s == steps - 1)
                    # hidT = tanh(W1n' @ hnT + b1)
                    for f in range(FD):
                        p1 = ps1p.tile([128, 512], f32, tag="ps1",
                                       name=f"p1_{s}_{f}")
                        for k in range(KD):
                            nc.tensor.matmul(
                                p1[:], w1[k][:, f * 128:(f + 1) * 128],
                                hnT[k][:],
                                start=(k == 0), stop=(k == KD - 1))
                        nc.scalar.activation(
                            hidT[f][:], p1[:], AF.Tanh, bias=b1s[:, f:f + 1])

                    if s == 0:
                        # w2/hwt loads deferred past step-0 mm1 so w1 gets
                        # full DMA bandwidth at startup; w2 arrives during
                        # mm1 execution, well before mm2 needs it
                        for f_ in range(FD):
                            nc.sync.dma_start(
                                w2[f_][:],
                                w2t_d.ap()[f_ * 128:(f_ + 1) * 128, :])
                        for k_ in range(KD):
                            nc.sync.dma_start(
                                hwt[k_][:],
                                hwt_d.ap()[k_ * 128:(k_ + 1) * 128, :])

                    # per row-tile: matmul2 (+xeg seeded in PSUM), update,
                    # stats, rstd, normalize, transpose -- interleaved so PE
                    # never idles at the step boundary.
                    mv = stp.tile([128, RD * 2], f32, tag="mv",
                                  name=f"mv_{s}")
                    mvv = mv[:].rearrange("p (r x) -> p r x", x=2)
                    rs = stp.tile([128, RD], f32, tag="rs", name=f"rs_{s}")
                    hns = {}
                    for r in range(RD):
                        st6 = stp.tile([128, 12], f32, tag="st6",
                                       name=f"st6_{s}_{r}")
                        for half in range(NH):
                            sl = slice(half * 512, (half + 1) * 512)
                            p2 = ps2p.tile([128, 512], f32, tag="ps2",
                                           name=f"p2_{s}_{r}_{half}")
                            nc.tensor.matmul(
                                p2[:], ident[:], xeg[r][:, sl],
                                start=True, stop=False)
                            for f in range(FD):
                                nc.tensor.matmul(
                                    p2[:], hidT[f][:, r * 128:(r + 1) * 128],
                                    w2[f][:, sl],
                                    start=False, stop=(f == FD - 1))
                            nc.vector.scalar_tensor_tensor(
                                h[r][:, sl], h[r][:, sl], 1.0 - GAMMA, p2[:],
                                op0=OP.mult, op1=OP.add)
                            if not last:
                                # stats chunk for this half right away
                                nc.vector.bn_stats(
                                    st6[:, half * 6:(half + 1) * 6],
                                    h[r][:, sl])
                        if last:
                            # head prep inline: cast final h to fp16 so its
                            # transposes overlap the remaining matmul2 groups
                            hc16 = wk.tile([128, DH], f16, tag=f"hn16_{r}",
                                           name=f"hc16_{r}", bufs=1)
                            nc.vector.tensor_copy(hc16[:], h[r][:])
                            hns[r] = hc16
                            continue
                        nc.vector.bn_aggr(mvv[:, r], st6[:])
                        rstd_newton(rs[:, r:r + 1], rs_p[:, r:r + 1],
                                    mv[:, 2 * r + 1:2 * r + 2], f"{s}_{r}")
                        hns[r] = normalize(r, mv[:, 2 * r:2 * r + 1],
                                           rs[:, r:r + 1], f"{s}_{r}")
                    # transposes LAST: PE has cover work while the final
                    # row-tile's DVE chain drains, so it never idles
                    for r in range(RD):
                        transposes(r, hns[r], f"{s}_{r}")
                    mv_p, rs_p = mv, rs

                # ---- head: outT = head_w @ h.T + head_b ----
                # (hnT already holds final h transposed, prepped in-loop)
                for ot in range(OD):
                    po = ps1p.tile([128, 512], f32, tag="ps1", name=f"po_{ot}")
                    for k in range(KD):
                        nc.tensor.matmul(
                            po[:], hwt[k][:, ot * 128:(ot + 1) * 128],
                            hnT[k][:],
                            start=(k == 0), stop=(k == KD - 1))
                    osb = wk.tile([128, 512], f32, tag="osb",
                                  name=f"osb_{ot}", bufs=1)
                    nc.scalar.activation(
                        osb[:], po[:], AF.Identity, bias=hbs[:, ot:ot + 1])
                    nc.sync.dma_start(
                        outT_d.ap()[ot * 128:(ot + 1) * 128, :], osb[:])

    nc.compile()
    return nc


def _get_compiled(steps: int):
    key = ("prog", steps)
    if key not in _CACHE:
        from concourse.bass_interp import get_hw_module
        nc = _build_program(steps)
        nc.m = get_hw_module(nc.m)
        _CACHE[key] = nc
    return _CACHE[key]


def _spectral_sigma(W: np.ndarray) -> float:
    W = W.astype(np.float64)
    v = np.full(W.shape[1], 1.0 / np.sqrt(W.shape[1]))
    u = W @ v
    u = u / (np.linalg.norm(u) + 1e-12)
    for _ in range(15):
        u = W @ v
        u = u / (np.linalg.norm(u) + 1e-12)
        v = W.T @ u
        v = v / (np.linalg.norm(v) + 1e-12)
    return float(u @ (W @ v))


def _prep_host(inputs: dict) -> tuple[dict, list]:
    f = {k: np.asarray(v, dtype=np.float32) for k, v in inputs.items()}
    x, ew, eb = f["x"], f["embed_w"], f["embed_b"]
    W1, b1, W2, b2 = f["W1"], f["b1"], f["W2"], f["b2"]
    ln_g, ln_b = f["ln_g"], f["ln_b"]
    hw_, hb = f["head_w"], f["head_b"]

    s1 = _spectral_sigma(W1)
    s2 = _spectral_sigma(W2)
    W1n = (W1.astype(np.float64) / s1)
    W2n = (W2.astype(np.float64) / s2)
    # fold ln gain into W1, ln bias into b1
    W1eff = W1n * ln_g.astype(np.float64)[None, :]
    b1eff = (b1.astype(np.float64) + W1n @ ln_b.astype(np.float64))
    W2eff = GAMMA * W2n

    shared = {
        "ewT": np.ascontiguousarray(ew.T).astype(np.float16),
        "w1t": np.ascontiguousarray(W1eff.T).astype(np.float16),
        "b1e": b1eff.astype(np.float32).reshape(DFF, 1),
        "w2t": np.ascontiguousarray(W2eff.T).astype(np.float16),
        "vb1": eb.reshape(1, DH).astype(np.float32),
        "vb2": (GAMMA * b2).reshape(1, DH).astype(np.float32),
        "hwt": np.ascontiguousarray(hw_.T).astype(np.float16),
        "hb": hb.reshape(DOUT, 1).astype(np.float32),
    }
    in_maps = []
    for c in range(N_CORES):
        shard = x[c * R:(c + 1) * R, :]
        m = dict(shared)
        m["xT"] = np.ascontiguousarray(shard.T).astype(np.float16)
        in_maps.append(m)
    return shared, in_maps


def kernel(**inputs) -> np.ndarray:
    from concourse import bass_utils
    nc = _get_compiled(STEPS)
    _, in_maps = _prep_host(inputs)
    res = None
    for attempt in range(3):
        try:
            res = bass_utils.run_bass_kernel_spmd(
                nc, in_maps, core_ids=list(range(N_CORES)))
            break
        except Exception:
            # transient NRT_EXEC_UNIT_UNRECOVERABLE device wedges clear on
            # retry
            if attempt == 2:
                raise
    out = np.empty((B, DOUT), np.float32)
    for c in range(N_CORES):
        out[c * R:(c + 1) * R, :] = res.results[c]["outT"].T
    return out


if __name__ == "__main__":
    rng = np.random.default_rng(0)
    demo = {
        "x": rng.standard_normal((B, DIN)).astype(np.float32),
        "embed_w": (rng.standard_normal((DH, DIN)) * 0.02).astype(np.float32),
        "embed_b": np.zeros(DH, np.float32),
        "W1": (rng.standard_normal((DFF, DH)) * 0.02).astype(np.float32),
        "b1": np.zeros(DFF, np.float32),
        "W2": (rng.standard_normal((DH, DFF)) * 0.02).astype(np.float32),
        "b2": np.zeros(DH, np.float32),
        "ln_g": np.ones(DH, np.float32),
        "ln_b": np.zeros(DH, np.float32),
        "head_w": (rng.standard_normal((DOUT, DH)) * 0.02).astype(np.float32),
        "head_b": np.zeros(DOUT, np.float32),
    }
    out = kernel(**demo)
    print("out", out.shape, out.dtype, float(np.abs(out).max()))



# revision 3
# speedup vs baseline: 2.0182x; 2.0182x over previous
"""Bass/Tile TRN2 kernel for nn_DimensionScaledEqProp.

Data-parallel over batch: x rows sharded across 8 NeuronCores, weights
replicated. Per-core state (h) stays resident in SBUF across the 30
sequential steps.

fp8(e4m3) DoubleRow matmuls for the two big GEMMs (2 MACs/cell/cycle),
fp32 state/accumulation. h is carried scaled by S2=256 so the fp8 weight
scaling folds away: LN is scale-invariant (eps scaled to match), and the
head weights absorb 1/S2.

Self-contained: hardcodes shapes; host side does sharding, spectral-norm
sigma (tiny: 60 matvecs), weight folding/quantization, and the final
gather/transpose.
"""
import sys
import numpy as np

for _p in ("/opt/trn_rl_repo", "/root/.axon_site/_ro/trn_rl_repo"):
    if _p not in sys.path:
        sys.path.append(_p)

B, DIN, DH, DOUT = 4096, 512, 1024, 256
DFF = 4 * DH
STEPS = 30
N_CORES = 8
R = B // N_CORES  # rows per core = 512
GAMMA = 0.5 * min(1.0, float(np.sqrt(64.0 / DIN)))
LN_EPS = 1e-5

KD = DH // 128    # 8  k-tiles over DH
FD = DFF // 128   # 32 f-tiles over DFF
RD = R // 128     # 4  row-tiles per core
ID = DIN // 128   # 4  k-tiles over DIN
OD = DOUT // 128  # 2  out-tiles over DOUT
NH = DH // 512    # 2  psum halves over DH

S1 = 64.0         # fp8 scale on W1eff (undone by tanh's activation scale)
S2 = 256.0        # fp8 scale on W2eff, carried inside h
SH = 16.0         # partial head-weight scale (rest via activation scale)
EPS_DEV = LN_EPS * S2 * S2

KP = KD // 2      # 4  DoubleRow k-pairs for mm1
FP = FD // 2      # 16 DoubleRow f-pairs for mm2

_CACHE = {}


def _build_program(steps: int):
    import concourse.bass as bass
    import concourse.bacc as bacc
    import concourse.mybir as mybir
    from concourse import tile, masks

    f8 = mybir.dt.float8e4
    f16 = mybir.dt.float16
    f32 = mybir.dt.float32
    AF = mybir.ActivationFunctionType
    OP = mybir.AluOpType
    DR = mybir.MatmulPerfMode.DoubleRow

    nc = bacc.Bacc("TRN2", target_bir_lowering=False, debug=False,
                   enable_asserts=True, num_devices=N_CORES)

    xT_d = nc.dram_tensor("xT", [DIN, R], f16, kind="ExternalInput")
    ewT_d = nc.dram_tensor("ewT", [DIN, DH], f16, kind="ExternalInput")
    w1t_d = nc.dram_tensor("w1t", [DH, DFF], f8, kind="ExternalInput")
    b1e_d = nc.dram_tensor("b1e", [DFF, 1], f32, kind="ExternalInput")
    w2t_d = nc.dram_tensor("w2t", [DFF, DH], f8, kind="ExternalInput")
    vb1_d = nc.dram_tensor("vb1", [1, DH], f32, kind="ExternalInput")
    vb2_d = nc.dram_tensor("vb2", [1, DH], f32, kind="ExternalInput")
    hwt_d = nc.dram_tensor("hwt", [DH, DOUT], f16, kind="ExternalInput")
    hb_d = nc.dram_tensor("hb", [DOUT, 1], f32, kind="ExternalInput")
    outT_d = nc.dram_tensor("outT", [DOUT, R], f32, kind="ExternalOutput")

    with tile.TileContext(nc) as tc:
        with (
            tc.tile_pool(name="wp", bufs=1) as wp,
            tc.tile_pool(name="sp", bufs=1) as sp,
            tc.tile_pool(name="wk", bufs=2) as wk,
            tc.tile_pool(name="stp", bufs=2) as stp,
            tc.tile_pool(name="pst", bufs=4, space="PSUM") as pst,
            tc.tile_pool(name="ps1", bufs=2, space="PSUM") as ps1p,
            tc.tile_pool(name="ps2", bufs=2, space="PSUM") as ps2p,
        ):
            # ---- persistent weights / constants ----
            # DoubleRow pair layout: [128, 2, free]; dim1 = second k-subtile
            w1 = [wp.tile([128, 2, DFF], f8, name=f"w1_{j}") for j in range(KP)]
            w2 = [wp.tile([128, 2, DH], f8, name=f"w2_{j}") for j in range(FP)]
            hwt = [wp.tile([128, DOUT], f16, name=f"hwt_{k}") for k in range(KD)]
            b1s = wp.tile([128, FD], f32, name="b1s")
            hbs = wp.tile([128, OD], f32, name="hbs")
            ident = wp.tile([128, 128], f16, name="ident")

            # ---- persistent state ----
            h = [sp.tile([128, DH], f32, name=f"h_{r}") for r in range(RD)]
            xeg = [sp.tile([128, DH], f16, name=f"xeg_{r}") for r in range(RD)]
            hnT = [sp.tile([128, 2, R], f8, name=f"hnT_{j}") for j in range(KP)]
            hfT = [sp.tile([128, R], f16, name=f"hfT_{k}") for k in range(KD)]

            masks.make_identity(nc, ident[:])

            # ---- embed (transient pool, released before the step loop) ----
            with tc.tile_pool(name="ep", bufs=1) as ep:
                xts = [ep.tile([128, R], f16, name=f"xts_{i}")
                       for i in range(ID)]
                ewt = [ep.tile([128, DH], f16, name=f"ewt_{i}")
                       for i in range(ID)]
                bc1 = ep.tile([128, DH], f32, name="bc1")
                bc2 = ep.tile([128, DH], f32, name="bc2")
                for i in range(ID):
                    nc.sync.dma_start(
                        xts[i][:], xT_d.ap()[i * 128:(i + 1) * 128, :])
                    nc.sync.dma_start(
                        ewt[i][:], ewT_d.ap()[i * 128:(i + 1) * 128, :])
                nc.sync.dma_start(bc1[0:1, :], vb1_d.ap())
                nc.sync.dma_start(bc2[0:1, :], vb2_d.ap())
                nc.gpsimd.partition_broadcast(bc1[:], bc1[0:1, :])
                nc.gpsimd.partition_broadcast(bc2[:], bc2[0:1, :])

                # weight loads AFTER embed inputs: embed matmuls start
                # immediately; w1/w2 stream in behind them
                for j in range(KP):
                    for i2 in range(2):
                        k = 2 * j + i2
                        nc.sync.dma_start(
                            w1[j][:, i2, :],
                            w1t_d.ap()[k * 128:(k + 1) * 128, :])
                nc.sync.dma_start(
                    b1s[:], b1e_d.ap().rearrange("(f p) o -> p (f o)", p=128))
                nc.sync.dma_start(
                    hbs[:], hb_d.ap().rearrange("(t p) o -> p (t o)", p=128))

                # h0 = x @ ewT + embed_b ; xeg = g*h0 + g*S2*b2 (f16)
                for r in range(RD):
                    for half in range(NH):
                        sl = slice(half * 512, (half + 1) * 512)
                        pe = ps1p.tile([128, 512], f32, tag="ps1", name="pe")
                        for i in range(ID):
                            nc.tensor.matmul(
                                pe[:], xts[i][:, r * 128:(r + 1) * 128],
                                ewt[i][:, sl],
                                start=(i == 0), stop=(i == ID - 1))
                        nc.vector.tensor_tensor(
                            h[r][:, sl], pe[:], bc1[:, sl], op=OP.add)
                        nc.vector.scalar_tensor_tensor(
                            xeg[r][:, sl], h[r][:, sl], GAMMA, bc2[:, sl],
                            op0=OP.mult, op1=OP.add)

            # ---- initial LN stats on h0 (ACT sqrt once; hides table load) ----
            mv0 = stp.tile([128, RD * 2], f32, tag="mv", name="mv_init")
            for r in range(RD):
                st6 = stp.tile([128, 12], f32, tag="st6", name=f"st6_i_{r}")
                for c in range(2):
                    nc.vector.bn_stats(
                        st6[:, c * 6:(c + 1) * 6],
                        h[r][:, c * 512:(c + 1) * 512])
                nc.vector.bn_aggr(
                    mv0[:].rearrange("p (r x) -> p r x", x=2)[:, r], st6[:])
            ve0 = stp.tile([128, RD], f32, tag="ve", name="ve_init")
            nc.vector.tensor_scalar(
                ve0[:], mv0[:].rearrange("p (r x) -> p x r", x=2)[:, 1],
                EPS_DEV, None, op0=OP.add)
            rv0 = stp.tile([128, RD], f32, tag="rv", name="rv_init")
            nc.vector.reciprocal(rv0[:], ve0[:])
            rs_prev = stp.tile([128, RD], f32, tag="rs", name="rs_init")
            nc.scalar.activation(rs_prev[:], rv0[:], AF.Sqrt)

            # ---- hidT pool reuses the embed pool space ----
            with tc.tile_pool(name="hp", bufs=1) as hp:
                hidT = [hp.tile([128, 2, R], f8, name=f"hidT_{j}")
                        for j in range(FP)]

                def rstd_newton(y_out, y_seed, var_ap, tag_sfx, n_iter=2):
                    """y_out[128,1] = 1/sqrt(var+eps) via Newton from seed."""
                    hv = stp.tile([128, 1], f32, tag="hv",
                                  name=f"hv_{tag_sfx}")
                    nc.vector.tensor_scalar(
                        hv[:], var_ap, -0.5, -0.5 * EPS_DEV,
                        op0=OP.mult, op1=OP.add)
                    y = y_seed
                    for it in range(n_iter):
                        a = stp.tile([128, 1], f32, tag="nwa",
                                     name=f"nwa_{tag_sfx}_{it}")
                        nc.vector.tensor_tensor(a[:], y, y, op=OP.mult)
                        nc.vector.tensor_scalar(
                            a[:], a[:], hv[:], 1.5, op0=OP.mult, op1=OP.add)
                        if it == n_iter - 1:
                            nc.vector.tensor_tensor(y_out, y, a[:], op=OP.mult)
                        else:
                            yn = stp.tile([128, 1], f32, tag="nwy",
                                          name=f"nwy_{tag_sfx}_{it}")
                            nc.vector.tensor_tensor(yn[:], y, a[:], op=OP.mult)
                            y = yn[:]

                # normalize h0 -> hnT for step 0 (one Newton polish on seed)
                rs_fix = stp.tile([128, RD], f32, tag="rsf", name="rs_fix")
                for r in range(RD):
                    rstd_newton(rs_fix[:, r:r + 1], rs_prev[:, r:r + 1],
                                mv0[:, 2 * r + 1:2 * r + 2], f"i{r}", n_iter=1)
                rs_p = rs_fix

                def normalize(r, mean_ap, rs_col, sfx):
                    nmu = stp.tile([128, 1], f32, tag="nmu", name=f"nmu_{sfx}")
                    nc.vector.scalar_tensor_tensor(
                        nmu[:], mean_ap, -1.0, rs_col,
                        op0=OP.mult, op1=OP.mult)
                    hn16 = wk.tile([128, DH], f16, tag=f"hn16_{r}",
                                   name=f"hn16_{sfx}", bufs=1)
                    # two half-width ops so transposes of the low half can
                    # start before the high half is normalized
                    for half in range(NH):
                        sl = slice(half * 512, (half + 1) * 512)
                        nc.vector.tensor_scalar(
                            hn16[:, sl], h[r][:, sl], rs_col, nmu[:],
                            op0=OP.mult, op1=OP.add)
                    return hn16

                def transposes(r, hn16, sfx):
                    # f16 PE transpose; PSUM->SBUF copy casts to fp8 pairs
                    for k in range(KD):
                        tp = pst.tile([128, 128], f16, tag="tp",
                                      name=f"tp_{sfx}_{k}")
                        nc.tensor.transpose(
                            tp[:], hn16[:, k * 128:(k + 1) * 128], ident[:])
                        dst = hnT[k // 2][:, k % 2, r * 128:(r + 1) * 128]
                        if k % 2 == 0:
                            nc.vector.tensor_copy(dst, tp[:])
                        else:
                            nc.scalar.copy(dst, tp[:])

                for r in range(RD):
                    hn = normalize(r, mv0[:, 2 * r:2 * r + 1],
                                   rs_p[:, r:r + 1], f"s0_{r}")
                    transposes(r, hn, f"s0_{r}")

                for s in range(steps):
                    last = (s == steps - 1)
                    # mm1: hidT = tanh((W1s' @ hnT)/S1 + b1)  [DoubleRow fp8]
                    for f in range(FD):
                        p1 = ps1p.tile([128, 512], f32, tag="ps1",
                                       name=f"p1_{s}_{f}")
                        for j in range(KP):
                            nc.tensor.matmul(
                                p1[:], w1[j][:, :, f * 128:(f + 1) * 128],
                                hnT[j][:],
                                start=(j == 0), stop=(j == KP - 1),
                                perf_mode=DR)
                        nc.scalar.activation(
                            hidT[f // 2][:, f % 2, :], p1[:], AF.Tanh,
                            bias=b1s[:, f:f + 1], scale=1.0 / S1)

                    if s == 0:
                        # w2/hwt loads deferred past step-0 mm1 so w1 gets
                        # full DMA bandwidth at startup; w2 arrives during
                        # mm1 execution, well before mm2 needs it
                        for j in range(FP):
                            for i2 in range(2):
                                f_ = 2 * j + i2
                                nc.sync.dma_start(
                                    w2[j][:, i2, :],
                                    w2t_d.ap()[f_ * 128:(f_ + 1) * 128, :])
                        for k_ in range(KD):
                            nc.sync.dma_start(
                                hwt[k_][:],
                                hwt_d.ap()[k_ * 128:(k_ + 1) * 128, :])

                    # pre-add (DVE, overlaps mm1): h = (1-g)*h + xeg
                    for r in range(RD):
                        nc.vector.scalar_tensor_tensor(
                            h[r][:], h[r][:], 1.0 - GAMMA, xeg[r][:],
                            op0=OP.mult, op1=OP.add)

                    # mm2 per (row-tile, half): p2 = W2s' @ hidT [DoubleRow],
                    # then h += p2; stats/rstd/normalize/transpose interleaved
                    mv = stp.tile([128, RD * 2], f32, tag="mv",
                                  name=f"mv_{s}")
                    mvv = mv[:].rearrange("p (r x) -> p r x", x=2)
                    rs = stp.tile([128, RD], f32, tag="rs", name=f"rs_{s}")
                    hns = {}
                    for r in range(RD):
                        st6 = stp.tile([128, 12], f32, tag="st6",
                                       name=f"st6_{s}_{r}")
                        for half in range(NH):
                            sl = slice(half * 512, (half + 1) * 512)
                            p2 = ps2p.tile([128, 512], f32, tag="ps2",
                                           name=f"p2_{s}_{r}_{half}")
                            for j in range(FP):
                                nc.tensor.matmul(
                                    p2[:],
                                    hidT[j][:, :, r * 128:(r + 1) * 128],
                                    w2[j][:, :, sl],
                                    start=(j == 0), stop=(j == FP - 1),
                                    perf_mode=DR)
                            nc.vector.tensor_tensor(
                                h[r][:, sl], h[r][:, sl], p2[:], op=OP.add)
                            if not last:
                                nc.vector.bn_stats(
                                    st6[:, half * 6:(half + 1) * 6],
                                    h[r][:, sl])
                        if last:
                            # head prep inline: cast final h to fp16 so its
                            # transposes overlap the remaining matmul2 groups
                            hc16 = wk.tile([128, DH], f16, tag=f"hn16_{r}",
                                           name=f"hc16_{r}", bufs=1)
                            nc.vector.tensor_copy(hc16[:], h[r][:])
                            hns[r] = hc16
                            continue
                        nc.vector.bn_aggr(mvv[:, r], st6[:])
                        rstd_newton(rs[:, r:r + 1], rs_p[:, r:r + 1],
                                    mv[:, 2 * r + 1:2 * r + 2], f"{s}_{r}")
                        hns[r] = normalize(r, mv[:, 2 * r:2 * r + 1],
                                           rs[:, r:r + 1], f"{s}_{r}")
                    # transposes LAST: PE has cover work while the final
                    # row-tile's DVE chain drains, so it never idles
                    if last:
                        for r in range(RD):
                            for k in range(KD):
                                tp = pst.tile([128, 128], f16, tag="tp",
                                              name=f"tpf_{r}_{k}")
                                nc.tensor.transpose(
                                    tp[:],
                                    hns[r][:, k * 128:(k + 1) * 128],
                                    ident[:])
                                dst = hfT[k][:, r * 128:(r + 1) * 128]
                                if k % 2 == 0:
                                    nc.vector.tensor_copy(dst, tp[:])
                                else:
                                    nc.scalar.copy(dst, tp[:])
                    else:
                        for r in range(RD):
                            transposes(r, hns[r], f"{s}_{r}")
                    rs_p = rs

                # ---- head: outT = (head_w/SH) @ h.T, out = po/SH' + hb ----
                for ot in range(OD):
                    po = ps1p.tile([128, 512], f32, tag="ps1", name=f"po_{ot}")
                    for k in range(KD):
                        nc.tensor.matmul(
                            po[:], hwt[k][:, ot * 128:(ot + 1) * 128],
                            hfT[k][:],
                            start=(k == 0), stop=(k == KD - 1))
                    osb = wk.tile([128, 512], f32, tag="osb",
                                  name=f"osb_{ot}", bufs=1)
                    nc.scalar.activation(
                        osb[:], po[:], AF.Identity, bias=hbs[:, ot:ot + 1],
                        scale=SH / S2)
                    nc.sync.dma_start(
                        outT_d.ap()[ot * 128:(ot + 1) * 128, :], osb[:])

    nc.compile()
    return nc


def _get_compiled(steps: int):
    key = ("prog", steps)
    if key not in _CACHE:
        from concourse.bass_interp import get_hw_module
        nc = _build_program(steps)
        nc.m = get_hw_module(nc.m)
        _CACHE[key] = nc
    return _CACHE[key]


def _spectral_sigma(W: np.ndarray) -> float:
    W = W.astype(np.float64)
    v = np.full(W.shape[1], 1.0 / np.sqrt(W.shape[1]))
    u = W @ v
    u = u / (np.linalg.norm(u) + 1e-12)
    for _ in range(15):
        u = W @ v
        u = u / (np.linalg.norm(u) + 1e-12)
        v = W.T @ u
        v = v / (np.linalg.norm(v) + 1e-12)
    return float(u @ (W @ v))


def _q8(x: np.ndarray) -> np.ndarray:
    import ml_dtypes
    return np.ascontiguousarray(
        np.clip(x, -240.0, 240.0).astype(ml_dtypes.float8_e4m3))


def _prep_host(inputs: dict) -> tuple[dict, list]:
    f = {k: np.asarray(v, dtype=np.float32) for k, v in inputs.items()}
    x, ew, eb = f["x"], f["embed_w"], f["embed_b"]
    W1, b1, W2, b2 = f["W1"], f["b1"], f["W2"], f["b2"]
    ln_g, ln_b = f["ln_g"], f["ln_b"]
    hw_, hb = f["head_w"], f["head_b"]

    s1 = _spectral_sigma(W1)
    s2 = _spectral_sigma(W2)
    W1n = (W1.astype(np.float64) / s1)
    W2n = (W2.astype(np.float64) / s2)
    # fold ln gain into W1, ln bias into b1
    W1eff = W1n * ln_g.astype(np.float64)[None, :]
    b1eff = (b1.astype(np.float64) + W1n @ ln_b.astype(np.float64))
    W2eff = GAMMA * W2n

    shared = {
        "ewT": np.ascontiguousarray(S2 * ew.T).astype(np.float16),
        "w1t": _q8(np.ascontiguousarray(S1 * W1eff.T)),
        "b1e": b1eff.astype(np.float32).reshape(DFF, 1),
        "w2t": _q8(np.ascontiguousarray(S2 * W2eff.T)),
        "vb1": (S2 * eb).reshape(1, DH).astype(np.float32),
        "vb2": (S2 * GAMMA * b2).reshape(1, DH).astype(np.float32),
        "hwt": np.ascontiguousarray(hw_.T / SH).astype(np.float16),
        "hb": hb.reshape(DOUT, 1).astype(np.float32),
    }
    in_maps = []
    for c in range(N_CORES):
        shard = x[c * R:(c + 1) * R, :]
        m = dict(shared)
        m["xT"] = np.ascontiguousarray(shard.T).astype(np.float16)
        in_maps.append(m)
    return shared, in_maps


def kernel(**inputs) -> np.ndarray:
    from concourse import bass_utils
    nc = _get_compiled(STEPS)
    _, in_maps = _prep_host(inputs)
    res = None
    for attempt in range(3):
        try:
            res = bass_utils.run_bass_kernel_spmd(
                nc, in_maps, core_ids=list(range(N_CORES)))
            break
        except Exception:
            # transient NRT_EXEC_UNIT_UNRECOVERABLE device wedges clear on
            # retry
            if attempt == 2:
                raise
    out = np.empty((B, DOUT), np.float32)
    for c in range(N_CORES):
        out[c * R:(c + 1) * R, :] = res.results[c]["outT"].T
    return out


if __name__ == "__main__":
    rng = np.random.default_rng(0)
    demo = {
        "x": rng.standard_normal((B, DIN)).astype(np.float32),
        "embed_w": (rng.standard_normal((DH, DIN)) * 0.02).astype(np.float32),
        "embed_b": np.zeros(DH, np.float32),
        "W1": (rng.standard_normal((DFF, DH)) * 0.02).astype(np.float32),
        "b1": np.zeros(DFF, np.float32),
        "W2": (rng.standard_normal((DH, DFF)) * 0.02).astype(np.float32),
        "b2": np.zeros(DH, np.float32),
        "ln_g": np.ones(DH, np.float32),
        "ln_b": np.zeros(DH, np.float32),
        "head_w": (rng.standard_normal((DOUT, DH)) * 0.02).astype(np.float32),
        "head_b": np.zeros(DOUT, np.float32),
    }
    out = kernel(**demo)
    print("out", out.shape, out.dtype, float(np.abs(out).max()))


# revision 8
# speedup vs baseline: 2.0238x; 1.0028x over previous
"""Bass/Tile TRN2 kernel for nn_DimensionScaledEqProp.

Data-parallel over batch: x rows sharded across 8 NeuronCores, weights
replicated. Per-core state (h) stays resident in SBUF across the 30
sequential steps.

fp8(e4m3) DoubleRow matmuls for the two big GEMMs (2 MACs/cell/cycle),
fp32 state/accumulation. h is carried scaled by S2=256 so the fp8 weight
scaling folds away: LN is scale-invariant (eps scaled to match), and the
head weights absorb 1/S2.

Self-contained: hardcodes shapes; host side does sharding, spectral-norm
sigma (tiny: 60 matvecs), weight folding/quantization, and the final
gather/transpose.
"""
import sys
import numpy as np

for _p in ("/opt/trn_rl_repo", "/root/.axon_site/_ro/trn_rl_repo"):
    if _p not in sys.path:
        sys.path.append(_p)

B, DIN, DH, DOUT = 4096, 512, 1024, 256
DFF = 4 * DH
STEPS = 30
N_CORES = 8
R = B // N_CORES  # rows per core = 512
GAMMA = 0.5 * min(1.0, float(np.sqrt(64.0 / DIN)))
LN_EPS = 1e-5

KD = DH // 128    # 8  k-tiles over DH
FD = DFF // 128   # 32 f-tiles over DFF
RD = R // 128     # 4  row-tiles per core
ID = DIN // 128   # 4  k-tiles over DIN
OD = DOUT // 128  # 2  out-tiles over DOUT
NH = DH // 512    # 2  psum halves over DH

S1 = 64.0         # fp8 scale on W1eff (undone by tanh's activation scale)
S2 = 256.0        # fp8 scale on W2eff, carried inside h
SH = 16.0         # partial head-weight scale (rest via activation scale)
EPS_DEV = LN_EPS * S2 * S2

KP = KD // 2      # 4  DoubleRow k-pairs for mm1
FP = FD // 2      # 16 DoubleRow f-pairs for mm2

_CACHE = {}


def _build_program(steps: int):
    import concourse.bass as bass
    import concourse.bacc as bacc
    import concourse.mybir as mybir
    from concourse import tile, masks

    f8 = mybir.dt.float8e4
    f16 = mybir.dt.float16
    f32 = mybir.dt.float32
    AF = mybir.ActivationFunctionType
    OP = mybir.AluOpType
    DR = mybir.MatmulPerfMode.DoubleRow

    nc = bacc.Bacc("TRN2", target_bir_lowering=False, debug=False,
                   enable_asserts=True, num_devices=N_CORES)

    xT_d = nc.dram_tensor("xT", [DIN, R], f16, kind="ExternalInput")
    ewT_d = nc.dram_tensor("ewT", [DIN, DH], f16, kind="ExternalInput")
    w1t_d = nc.dram_tensor("w1t", [DH, DFF], f8, kind="ExternalInput")
    b1e_d = nc.dram_tensor("b1e", [DFF, 1], f32, kind="ExternalInput")
    w2t_d = nc.dram_tensor("w2t", [DFF, DH], f8, kind="ExternalInput")
    vb1_d = nc.dram_tensor("vb1", [1, DH], f32, kind="ExternalInput")
    vb2_d = nc.dram_tensor("vb2", [1, DH], f32, kind="ExternalInput")
    hwt_d = nc.dram_tensor("hwt", [DH, DOUT], f16, kind="ExternalInput")
    hb_d = nc.dram_tensor("hb", [DOUT, 1], f32, kind="ExternalInput")
    outT_d = nc.dram_tensor("outT", [DOUT, R], f32, kind="ExternalOutput")

    with tile.TileContext(nc) as tc:
        with (
            tc.tile_pool(name="wp", bufs=1) as wp,
            tc.tile_pool(name="sp", bufs=1) as sp,
            tc.tile_pool(name="wk", bufs=2) as wk,
            tc.tile_pool(name="stp", bufs=2) as stp,
            tc.tile_pool(name="pst", bufs=4, space="PSUM") as pst,
            tc.tile_pool(name="ps1", bufs=2, space="PSUM") as ps1p,
            tc.tile_pool(name="ps2", bufs=2, space="PSUM") as ps2p,
        ):
            # ---- persistent weights / constants ----
            # DoubleRow pair layout: [128, 2, free]; dim1 = second k-subtile
            w1 = [wp.tile([128, 2, DFF], f8, name=f"w1_{j}") for j in range(KP)]
            w2 = [wp.tile([128, 2, DH], f8, name=f"w2_{j}") for j in range(FP)]
            hwt = [wp.tile([128, DOUT], f16, name=f"hwt_{k}") for k in range(KD)]
            b1s = wp.tile([128, FD], f32, name="b1s")
            hbs = wp.tile([128, OD], f32, name="hbs")
            ident = wp.tile([128, 128], f16, name="ident")

            # ---- persistent state ----
            h = [sp.tile([128, DH], f32, name=f"h_{r}") for r in range(RD)]
            xeg = [sp.tile([128, DH], f16, name=f"xeg_{r}") for r in range(RD)]
            hnT = [sp.tile([128, 2, R], f8, name=f"hnT_{j}") for j in range(KP)]
            hfT = [sp.tile([128, 2, R], f16, name=f"hfT_{j}")
                   for j in range(KP)]

            masks.make_identity(nc, ident[:])

            # ---- embed (transient pool, released before the step loop) ----
            with tc.tile_pool(name="ep", bufs=1) as ep:
                xts = [ep.tile([128, R], f16, name=f"xts_{i}")
                       for i in range(ID)]
                ewt = [ep.tile([128, DH], f16, name=f"ewt_{i}")
                       for i in range(ID)]
                bc1 = ep.tile([128, DH], f32, name="bc1")
                bc2 = ep.tile([128, DH], f32, name="bc2")
                for i in range(ID):
                    nc.sync.dma_start(
                        xts[i][:], xT_d.ap()[i * 128:(i + 1) * 128, :])
                    nc.sync.dma_start(
                        ewt[i][:], ewT_d.ap()[i * 128:(i + 1) * 128, :])
                nc.sync.dma_start(bc1[0:1, :], vb1_d.ap())
                nc.sync.dma_start(bc2[0:1, :], vb2_d.ap())
                nc.gpsimd.partition_broadcast(bc1[:], bc1[0:1, :])
                nc.gpsimd.partition_broadcast(bc2[:], bc2[0:1, :])

                # weight loads AFTER embed inputs: embed matmuls start
                # immediately; w1/w2 stream in behind them
                for j in range(KP):
                    for i2 in range(2):
                        k = 2 * j + i2
                        nc.sync.dma_start(
                            w1[j][:, i2, :],
                            w1t_d.ap()[k * 128:(k + 1) * 128, :])
                nc.sync.dma_start(
                    b1s[:], b1e_d.ap().rearrange("(f p) o -> p (f o)", p=128))
                nc.sync.dma_start(
                    hbs[:], hb_d.ap().rearrange("(t p) o -> p (t o)", p=128))

                # h0 = x @ ewT + embed_b ; xeg = g*h0 + g*S2*b2 (f16)
                for r in range(RD):
                    for half in range(NH):
                        sl = slice(half * 512, (half + 1) * 512)
                        pe = ps1p.tile([128, 512], f32, tag="ps1", name="pe")
                        for i in range(ID):
                            nc.tensor.matmul(
                                pe[:], xts[i][:, r * 128:(r + 1) * 128],
                                ewt[i][:, sl],
                                start=(i == 0), stop=(i == ID - 1))
                        nc.vector.tensor_tensor(
                            h[r][:, sl], pe[:], bc1[:, sl], op=OP.add)
                        nc.vector.scalar_tensor_tensor(
                            xeg[r][:, sl], h[r][:, sl], GAMMA, bc2[:, sl],
                            op0=OP.mult, op1=OP.add)

            # ---- initial LN stats on h0 (ACT sqrt once; hides table load) ----
            mv0 = stp.tile([128, RD * 2], f32, tag="mv", name="mv_init")
            for r in range(RD):
                st6 = stp.tile([128, 12], f32, tag="st6", name=f"st6_i_{r}")
                for c in range(2):
                    nc.vector.bn_stats(
                        st6[:, c * 6:(c + 1) * 6],
                        h[r][:, c * 512:(c + 1) * 512])
                nc.vector.bn_aggr(
                    mv0[:].rearrange("p (r x) -> p r x", x=2)[:, r], st6[:])
            ve0 = stp.tile([128, RD], f32, tag="ve", name="ve_init")
            nc.vector.tensor_scalar(
                ve0[:], mv0[:].rearrange("p (r x) -> p x r", x=2)[:, 1],
                EPS_DEV, None, op0=OP.add)
            rv0 = stp.tile([128, RD], f32, tag="rv", name="rv_init")
            nc.vector.reciprocal(rv0[:], ve0[:])
            rs_prev = stp.tile([128, RD], f32, tag="rs", name="rs_init")
            nc.scalar.activation(rs_prev[:], rv0[:], AF.Sqrt)

            # ---- hidT pool reuses the embed pool space ----
            with tc.tile_pool(name="hp", bufs=1) as hp:
                hidT = [hp.tile([128, 2, R], f8, name=f"hidT_{j}")
                        for j in range(FP)]

                def rstd_newton(y_out, y_seed, var_ap, tag_sfx, n_iter=2):
                    """y_out[128,1] = 1/sqrt(var+eps) via Newton from seed."""
                    hv = stp.tile([128, 1], f32, tag="hv",
                                  name=f"hv_{tag_sfx}")
                    nc.vector.tensor_scalar(
                        hv[:], var_ap, -0.5, -0.5 * EPS_DEV,
                        op0=OP.mult, op1=OP.add)
                    y = y_seed
                    for it in range(n_iter):
                        a = stp.tile([128, 1], f32, tag="nwa",
                                     name=f"nwa_{tag_sfx}_{it}")
                        nc.vector.tensor_tensor(a[:], y, y, op=OP.mult)
                        nc.vector.tensor_scalar(
                            a[:], a[:], hv[:], 1.5, op0=OP.mult, op1=OP.add)
                        if it == n_iter - 1:
                            nc.vector.tensor_tensor(y_out, y, a[:], op=OP.mult)
                        else:
                            yn = stp.tile([128, 1], f32, tag="nwy",
                                          name=f"nwy_{tag_sfx}_{it}")
                            nc.vector.tensor_tensor(yn[:], y, a[:], op=OP.mult)
                            y = yn[:]

                # normalize h0 -> hnT for step 0 (one Newton polish on seed)
                rs_fix = stp.tile([128, RD], f32, tag="rsf", name="rs_fix")
                for r in range(RD):
                    rstd_newton(rs_fix[:, r:r + 1], rs_prev[:, r:r + 1],
                                mv0[:, 2 * r + 1:2 * r + 2], f"i{r}", n_iter=1)
                rs_p = rs_fix

                def normalize(r, mean_ap, rs_col, sfx):
                    nmu = stp.tile([128, 1], f32, tag="nmu", name=f"nmu_{sfx}")
                    nc.vector.scalar_tensor_tensor(
                        nmu[:], mean_ap, -1.0, rs_col,
                        op0=OP.mult, op1=OP.mult)
                    hn16 = wk.tile([128, DH], f16, tag=f"hn16_{r}",
                                   name=f"hn16_{sfx}", bufs=1)
                    # two half-width ops so transposes of the low half can
                    # start before the high half is normalized
                    for half in range(NH):
                        sl = slice(half * 512, (half + 1) * 512)
                        nc.vector.tensor_scalar(
                            hn16[:, sl], h[r][:, sl], rs_col, nmu[:],
                            op0=OP.mult, op1=OP.add)
                    return hn16

                def transposes(r, hn16, sfx):
                    # paired f16 PE transposes into one [128,256] PSUM tile;
                    # single PSUM->SBUF cast writes the fp8 DoubleRow pair
                    for j in range(KP):
                        tp = pst.tile([128, 256], f16, tag="tp",
                                      name=f"tp_{sfx}_{j}")
                        for i2 in range(2):
                            k = 2 * j + i2
                            nc.tensor.transpose(
                                tp[:, i2 * 128:(i2 + 1) * 128],
                                hn16[:, k * 128:(k + 1) * 128], ident[:])
                        dst = hnT[j][:, :, r * 128:(r + 1) * 128]
                        if j % 2 == 0:
                            nc.vector.tensor_copy(dst, tp[:])
                        else:
                            nc.scalar.copy(dst, tp[:])

                for r in range(RD):
                    hn = normalize(r, mv0[:, 2 * r:2 * r + 1],
                                   rs_p[:, r:r + 1], f"s0_{r}")
                    transposes(r, hn, f"s0_{r}")

                for s in range(steps):
                    last = (s == steps - 1)
                    # mm1: hidT = tanh((W1s' @ hnT)/S1 + b1)  [DoubleRow fp8]
                    for f in range(FD):
                        p1 = ps1p.tile([128, 512], f32, tag="ps1",
                                       name=f"p1_{s}_{f}")
                        for j in range(KP):
                            nc.tensor.matmul(
                                p1[:], w1[j][:, :, f * 128:(f + 1) * 128],
                                hnT[j][:],
                                start=(j == 0), stop=(j == KP - 1),
                                perf_mode=DR)
                        nc.scalar.activation(
                            hidT[f // 2][:, f % 2, :], p1[:], AF.Tanh,
                            bias=b1s[:, f:f + 1], scale=1.0 / S1)

                    if s == 0:
                        # w2/hwt loads deferred past step-0 mm1 so w1 gets
                        # full DMA bandwidth at startup; w2 arrives during
                        # mm1 execution, well before mm2 needs it
                        for j in range(FP):
                            for i2 in range(2):
                                f_ = 2 * j + i2
                                nc.sync.dma_start(
                                    w2[j][:, i2, :],
                                    w2t_d.ap()[f_ * 128:(f_ + 1) * 128, :])
                        for k_ in range(KD):
                            nc.sync.dma_start(
                                hwt[k_][:],
                                hwt_d.ap()[k_ * 128:(k_ + 1) * 128, :])

                    # pre-add (DVE, overlaps mm1): h = (1-g)*h + xeg
                    for r in range(RD):
                        nc.vector.scalar_tensor_tensor(
                            h[r][:], h[r][:], 1.0 - GAMMA, xeg[r][:],
                            op0=OP.mult, op1=OP.add)

                    # mm2 per (row-tile, half): p2 = W2s' @ hidT [DoubleRow],
                    # then h += p2; stats/rstd/normalize/transpose interleaved.
                    # transp(r0)/transp(r1) slot between later mm2 groups
                    # (their normalizes have drained by then); r2+r3 stay at
                    # the step end as PE cover while r3's DVE chain drains.
                    mv = stp.tile([128, RD * 2], f32, tag="mv",
                                  name=f"mv_{s}")
                    mvv = mv[:].rearrange("p (r x) -> p r x", x=2)
                    rs = stp.tile([128, RD], f32, tag="rs", name=f"rs_{s}")
                    hns = {}

                    def transp_any(r, sfx):
                        if last:
                            transposes_head(r, hns[r], sfx)
                        else:
                            transposes(r, hns[r], sfx)

                    def transposes_head(r, hn16, sfx):
                        for j in range(KP):
                            tp = pst.tile([128, 256], f16, tag="tp",
                                          name=f"tpf_{sfx}_{j}")
                            for i2 in range(2):
                                k = 2 * j + i2
                                nc.tensor.transpose(
                                    tp[:, i2 * 128:(i2 + 1) * 128],
                                    hn16[:, k * 128:(k + 1) * 128], ident[:])
                            dst = hfT[j][:, :, r * 128:(r + 1) * 128]
                            if j % 2 == 0:
                                nc.vector.tensor_copy(dst, tp[:])
                            else:
                                nc.scalar.copy(dst, tp[:])

                    for r in range(RD):
                        st6 = stp.tile([128, 12], f32, tag="st6",
                                       name=f"st6_{s}_{r}")
                        for half in range(NH):
                            sl = slice(half * 512, (half + 1) * 512)
                            p2 = ps2p.tile([128, 512], f32, tag="ps2",
                                           name=f"p2_{s}_{r}_{half}")
                            for j in range(FP):
                                nc.tensor.matmul(
                                    p2[:],
                                    hidT[j][:, :, r * 128:(r + 1) * 128],
                                    w2[j][:, :, sl],
                                    start=(j == 0), stop=(j == FP - 1),
                                    perf_mode=DR)
                            nc.vector.tensor_tensor(
                                h[r][:, sl], h[r][:, sl], p2[:], op=OP.add)
                            if not last:
                                nc.vector.bn_stats(
                                    st6[:, half * 6:(half + 1) * 6],
                                    h[r][:, sl])
                            if half == 0 and r >= 2:
                                # interleaved transposes of row r-2 (normalize
                                # long since drained; PE slots them between
                                # mm2 groups, casts go behind this row's stats)
                                transp_any(r - 2, f"{s}_{r - 2}")
                        if last:
                            # head prep inline: cast final h to fp16 so its
                            # transposes overlap the remaining matmul2 groups
                            hc16 = wk.tile([128, DH], f16, tag=f"hn16_{r}",
                                           name=f"hc16_{r}", bufs=1)
                            nc.vector.tensor_copy(hc16[:], h[r][:])
                            hns[r] = hc16
                            continue
                        nc.vector.bn_aggr(mvv[:, r], st6[:])
                        rstd_newton(rs[:, r:r + 1], rs_p[:, r:r + 1],
                                    mv[:, 2 * r + 1:2 * r + 2], f"{s}_{r}")
                        hns[r] = normalize(r, mv[:, 2 * r:2 * r + 1],
                                           rs[:, r:r + 1], f"{s}_{r}")
                    for r in (RD - 2, RD - 1):
                        transp_any(r, f"{s}_{r}")
                    rs_p = rs

                # ---- head: outT = (head_w/SH) @ h.T, out = po/SH' + hb ----
                for ot in range(OD):
                    po = ps1p.tile([128, 512], f32, tag="ps1", name=f"po_{ot}")
                    for k in range(KD):
                        nc.tensor.matmul(
                            po[:], hwt[k][:, ot * 128:(ot + 1) * 128],
                            hfT[k // 2][:, k % 2, :],
                            start=(k == 0), stop=(k == KD - 1))
                    osb = wk.tile([128, 512], f32, tag="osb",
                                  name=f"osb_{ot}", bufs=1)
                    nc.scalar.activation(
                        osb[:], po[:], AF.Identity, bias=hbs[:, ot:ot + 1],
                        scale=SH / S2)
                    nc.sync.dma_start(
                        outT_d.ap()[ot * 128:(ot + 1) * 128, :], osb[:])

    nc.compile()
    return nc


def _get_compiled(steps: int):
    key = ("prog", steps)
    if key not in _CACHE:
        from concourse.bass_interp import get_hw_module
        nc = _build_program(steps)
        nc.m = get_hw_module(nc.m)
        _CACHE[key] = nc
    return _CACHE[key]


def _spectral_sigma(W: np.ndarray) -> float:
    W = W.astype(np.float64)
    v = np.full(W.shape[1], 1.0 / np.sqrt(W.shape[1]))
    u = W @ v
    u = u / (np.linalg.norm(u) + 1e-12)
    for _ in range(15):
        u = W @ v
        u = u / (np.linalg.norm(u) + 1e-12)
        v = W.T @ u
        v = v / (np.linalg.norm(v) + 1e-12)
    return float(u @ (W @ v))


def _q8(x: np.ndarray) -> np.ndarray:
    import ml_dtypes
    return np.ascontiguousarray(
        np.clip(x, -240.0, 240.0).astype(ml_dtypes.float8_e4m3))


def _prep_host(inputs: dict) -> tuple[dict, list]:
    f = {k: np.asarray(v, dtype=np.float32) for k, v in inputs.items()}
    x, ew, eb = f["x"], f["embed_w"], f["embed_b"]
    W1, b1, W2, b2 = f["W1"], f["b1"], f["W2"], f["b2"]
    ln_g, ln_b = f["ln_g"], f["ln_b"]
    hw_, hb = f["head_w"], f["head_b"]

    s1 = _spectral_sigma(W1)
    s2 = _spectral_sigma(W2)
    W1n = (W1.astype(np.float64) / s1)
    W2n = (W2.astype(np.float64) / s2)
    # fold ln gain into W1, ln bias into b1
    W1eff = W1n * ln_g.astype(np.float64)[None, :]
    b1eff = (b1.astype(np.float64) + W1n @ ln_b.astype(np.float64))
    W2eff = GAMMA * W2n

    shared = {
        "ewT": np.ascontiguousarray(S2 * ew.T).astype(np.float16),
        "w1t": _q8(np.ascontiguousarray(S1 * W1eff.T)),
        "b1e": b1eff.astype(np.float32).reshape(DFF, 1),
        "w2t": _q8(np.ascontiguousarray(S2 * W2eff.T)),
        "vb1": (S2 * eb).reshape(1, DH).astype(np.float32),
        "vb2": (S2 * GAMMA * b2).reshape(1, DH).astype(np.float32),
        "hwt": np.ascontiguousarray(hw_.T / SH).astype(np.float16),
        "hb": hb.reshape(DOUT, 1).astype(np.float32),
    }
    in_maps = []
    for c in range(N_CORES):
        shard = x[c * R:(c + 1) * R, :]
        m = dict(shared)
        m["xT"] = np.ascontiguousarray(shard.T).astype(np.float16)
        in_maps.append(m)
    return shared, in_maps


def kernel(**inputs) -> np.ndarray:
    from concourse import bass_utils
    nc = _get_compiled(STEPS)
    _, in_maps = _prep_host(inputs)
    res = None
    for attempt in range(3):
        try:
            res = bass_utils.run_bass_kernel_spmd(
                nc, in_maps, core_ids=list(range(N_CORES)))
            break
        except Exception:
            # transient NRT_EXEC_UNIT_UNRECOVERABLE device wedges clear on
            # retry
            if attempt == 2:
                raise
    out = np.empty((B, DOUT), np.float32)
    for c in range(N_CORES):
        out[c * R:(c + 1) * R, :] = res.results[c]["outT"].T
    return out


if __name__ == "__main__":
    rng = np.random.default_rng(0)
    demo = {
        "x": rng.standard_normal((B, DIN)).astype(np.float32),
        "embed_w": (rng.standard_normal((DH, DIN)) * 0.02).astype(np.float32),
        "embed_b": np.zeros(DH, np.float32),
        "W1": (rng.standard_normal((DFF, DH)) * 0.02).astype(np.float32),
        "b1": np.zeros(DFF, np.float32),
        "W2": (rng.standard_normal((DH, DFF)) * 0.02).astype(np.float32),
        "b2": np.zeros(DH, np.float32),
        "ln_g": np.ones(DH, np.float32),
        "ln_b": np.zeros(DH, np.float32),
        "head_w": (rng.standard_normal((DOUT, DH)) * 0.02).astype(np.float32),
        "head_b": np.zeros(DOUT, np.float32),
    }
    out = kernel(**demo)
    print("out", out.shape, out.dtype, float(np.abs(out).max()))


# revision 10
# speedup vs baseline: 2.0730x; 1.0243x over previous
"""Bass/Tile TRN2 kernel for nn_DimensionScaledEqProp.

Data-parallel over batch: x rows sharded across 8 NeuronCores, weights
replicated. Per-core state (h) stays resident in SBUF across the 30
sequential steps.

fp8(e4m3) DoubleRow matmuls for the two big GEMMs (2 MACs/cell/cycle),
fp32 state/accumulation. h is carried scaled by S2=256 so the fp8 weight
scaling folds away: LN is scale-invariant (eps scaled to match), and the
head weights absorb 1/S2.

Self-contained: hardcodes shapes; host side does sharding, spectral-norm
sigma (tiny: 60 matvecs), weight folding/quantization, and the final
gather/transpose.
"""
import sys
import numpy as np

for _p in ("/opt/trn_rl_repo", "/root/.axon_site/_ro/trn_rl_repo"):
    if _p not in sys.path:
        sys.path.append(_p)

B, DIN, DH, DOUT = 4096, 512, 1024, 256
DFF = 4 * DH
STEPS = 30
N_CORES = 8
R = B // N_CORES  # rows per core = 512
GAMMA = 0.5 * min(1.0, float(np.sqrt(64.0 / DIN)))
LN_EPS = 1e-5

KD = DH // 128    # 8  k-tiles over DH
FD = DFF // 128   # 32 f-tiles over DFF
RD = R // 128     # 4  row-tiles per core
ID = DIN // 128   # 4  k-tiles over DIN
OD = DOUT // 128  # 2  out-tiles over DOUT
NH = DH // 512    # 2  psum halves over DH

S1 = 64.0         # fp8 scale on W1eff (undone by tanh's activation scale)
S2 = 256.0        # fp8 scale on W2eff, carried inside h
SH = 16.0         # partial head-weight scale (rest via activation scale)
EPS_DEV = LN_EPS * S2 * S2

KP = KD // 2      # 4  DoubleRow k-pairs for mm1
FP = FD // 2      # 16 DoubleRow f-pairs for mm2

_CACHE = {}


def _build_program(steps: int):
    import concourse.bass as bass
    import concourse.bacc as bacc
    import concourse.mybir as mybir
    from concourse import tile, masks

    f8 = mybir.dt.float8e4
    f16 = mybir.dt.float16
    f32 = mybir.dt.float32
    AF = mybir.ActivationFunctionType
    OP = mybir.AluOpType
    DR = mybir.MatmulPerfMode.DoubleRow

    nc = bacc.Bacc("TRN2", target_bir_lowering=False, debug=False,
                   enable_asserts=True, num_devices=N_CORES)

    xT_d = nc.dram_tensor("xT", [DIN, R], f16, kind="ExternalInput")
    ewT_d = nc.dram_tensor("ewT", [DIN, DH], f16, kind="ExternalInput")
    w1t_d = nc.dram_tensor("w1t", [DH, DFF], f8, kind="ExternalInput")
    b1e_d = nc.dram_tensor("b1e", [DFF, 1], f32, kind="ExternalInput")
    w2t_d = nc.dram_tensor("w2t", [DFF, DH], f8, kind="ExternalInput")
    vb1_d = nc.dram_tensor("vb1", [1, DH], f32, kind="ExternalInput")
    vb2_d = nc.dram_tensor("vb2", [1, DH], f32, kind="ExternalInput")
    hwt_d = nc.dram_tensor("hwt", [DH, DOUT], f16, kind="ExternalInput")
    hb_d = nc.dram_tensor("hb", [DOUT, 1], f32, kind="ExternalInput")
    outT_d = nc.dram_tensor("outT", [DOUT, R], f32, kind="ExternalOutput")

    with tile.TileContext(nc) as tc:
        with (
            tc.tile_pool(name="wp", bufs=1) as wp,
            tc.tile_pool(name="sp", bufs=1) as sp,
            tc.tile_pool(name="wk", bufs=2) as wk,
            tc.tile_pool(name="stp", bufs=2) as stp,
            tc.tile_pool(name="pst", bufs=4, space="PSUM") as pst,
            tc.tile_pool(name="ps1", bufs=2, space="PSUM") as ps1p,
            tc.tile_pool(name="ps2", bufs=2, space="PSUM") as ps2p,
        ):
            # ---- persistent weights / constants ----
            # DoubleRow pair layout: [128, 2, free]; dim1 = second k-subtile
            w1 = [wp.tile([128, 2, DFF], f8, name=f"w1_{j}") for j in range(KP)]
            w2 = [wp.tile([128, 2, DH], f8, name=f"w2_{j}") for j in range(FP)]
            hwt = [wp.tile([128, DOUT], f16, name=f"hwt_{k}") for k in range(KD)]
            b1s = wp.tile([128, FD], f32, name="b1s")
            hbs = wp.tile([128, OD], f32, name="hbs")
            ident = wp.tile([128, 128], f16, name="ident")

            # ---- persistent state ----
            h = [sp.tile([128, DH], f32, name=f"h_{r}") for r in range(RD)]
            xeg = [sp.tile([128, DH], f16, name=f"xeg_{r}") for r in range(RD)]
            hnT = [sp.tile([128, 2, R], f8, name=f"hnT_{j}") for j in range(KP)]
            hfT = [sp.tile([128, 2, R], f16, name=f"hfT_{j}")
                   for j in range(KP)]

            masks.make_identity(nc, ident[:])

            # ---- embed (transient pool, released before the step loop) ----
            with tc.tile_pool(name="ep", bufs=1) as ep:
                xts = [ep.tile([128, R], f16, name=f"xts_{i}")
                       for i in range(ID)]
                ewt = [ep.tile([128, DH], f16, name=f"ewt_{i}")
                       for i in range(ID)]
                bc1 = ep.tile([128, DH], f32, name="bc1")
                bc2 = ep.tile([128, DH], f32, name="bc2")
                for i in range(ID):
                    nc.sync.dma_start(
                        xts[i][:], xT_d.ap()[i * 128:(i + 1) * 128, :])
                    nc.sync.dma_start(
                        ewt[i][:], ewT_d.ap()[i * 128:(i + 1) * 128, :])
                nc.sync.dma_start(bc1[0:1, :], vb1_d.ap())
                nc.sync.dma_start(bc2[0:1, :], vb2_d.ap())
                nc.gpsimd.partition_broadcast(bc1[:], bc1[0:1, :])
                nc.gpsimd.partition_broadcast(bc2[:], bc2[0:1, :])

                # weight loads AFTER embed inputs: embed matmuls start
                # immediately; w1/w2 stream in behind them
                for j in range(KP):
                    for i2 in range(2):
                        k = 2 * j + i2
                        nc.sync.dma_start(
                            w1[j][:, i2, :],
                            w1t_d.ap()[k * 128:(k + 1) * 128, :])
                nc.sync.dma_start(
                    b1s[:], b1e_d.ap().rearrange("(f p) o -> p (f o)", p=128))
                nc.sync.dma_start(
                    hbs[:], hb_d.ap().rearrange("(t p) o -> p (t o)", p=128))

                # h0 = x @ ewT + embed_b ; xeg = g*h0 + g*S2*b2 (f16)
                for r in range(RD):
                    for half in range(NH):
                        sl = slice(half * 512, (half + 1) * 512)
                        pe = ps1p.tile([128, 512], f32, tag="ps1", name="pe")
                        for i in range(ID):
                            nc.tensor.matmul(
                                pe[:], xts[i][:, r * 128:(r + 1) * 128],
                                ewt[i][:, sl],
                                start=(i == 0), stop=(i == ID - 1))
                        nc.vector.tensor_tensor(
                            h[r][:, sl], pe[:], bc1[:, sl], op=OP.add)
                        nc.vector.scalar_tensor_tensor(
                            xeg[r][:, sl], h[r][:, sl], GAMMA, bc2[:, sl],
                            op0=OP.mult, op1=OP.add)

            # ---- initial LN stats on h0 (ACT sqrt once; hides table load) ----
            mv0 = stp.tile([128, RD * 2], f32, tag="mv", name="mv_init")
            for r in range(RD):
                st6 = stp.tile([128, 12], f32, tag="st6", name=f"st6_i_{r}")
                for c in range(2):
                    nc.vector.bn_stats(
                        st6[:, c * 6:(c + 1) * 6],
                        h[r][:, c * 512:(c + 1) * 512])
                nc.vector.bn_aggr(
                    mv0[:].rearrange("p (r x) -> p r x", x=2)[:, r], st6[:])
            ve0 = stp.tile([128, RD], f32, tag="ve", name="ve_init")
            nc.vector.tensor_scalar(
                ve0[:], mv0[:].rearrange("p (r x) -> p x r", x=2)[:, 1],
                EPS_DEV, None, op0=OP.add)
            rv0 = stp.tile([128, RD], f32, tag="rv", name="rv_init")
            nc.vector.reciprocal(rv0[:], ve0[:])
            rs_prev = stp.tile([128, RD], f32, tag="rs", name="rs_init")
            nc.scalar.activation(rs_prev[:], rv0[:], AF.Sqrt)

            # ---- hidT pool reuses the embed pool space ----
            with tc.tile_pool(name="hp", bufs=1) as hp:
                hidT = [hp.tile([128, 2, R], f8, name=f"hidT_{j}")
                        for j in range(FP)]

                def rstd_newton(y_out, y_seed, var_ap, tag_sfx, n_iter=2):
                    """y_out[128,1] = 1/sqrt(var+eps) via Newton from seed."""
                    hv = stp.tile([128, 1], f32, tag="hv",
                                  name=f"hv_{tag_sfx}")
                    nc.vector.tensor_scalar(
                        hv[:], var_ap, -0.5, -0.5 * EPS_DEV,
                        op0=OP.mult, op1=OP.add)
                    y = y_seed
                    for it in range(n_iter):
                        a = stp.tile([128, 1], f32, tag="nwa",
                                     name=f"nwa_{tag_sfx}_{it}")
                        nc.vector.tensor_tensor(a[:], y, y, op=OP.mult)
                        nc.vector.tensor_scalar(
                            a[:], a[:], hv[:], 1.5, op0=OP.mult, op1=OP.add)
                        if it == n_iter - 1:
                            nc.vector.tensor_tensor(y_out, y, a[:], op=OP.mult)
                        else:
                            yn = stp.tile([128, 1], f32, tag="nwy",
                                          name=f"nwy_{tag_sfx}_{it}")
                            nc.vector.tensor_tensor(yn[:], y, a[:], op=OP.mult)
                            y = yn[:]

                # normalize h0 -> hnT for step 0 (one Newton polish on seed)
                rs_fix = stp.tile([128, RD], f32, tag="rsf", name="rs_fix")
                for r in range(RD):
                    rstd_newton(rs_fix[:, r:r + 1], rs_prev[:, r:r + 1],
                                mv0[:, 2 * r + 1:2 * r + 2], f"i{r}", n_iter=1)
                rs_p = rs_fix

                def normalize(r, mean_ap, rs_col, sfx):
                    nmu = stp.tile([128, 1], f32, tag="nmu", name=f"nmu_{sfx}")
                    nc.vector.scalar_tensor_tensor(
                        nmu[:], mean_ap, -1.0, rs_col,
                        op0=OP.mult, op1=OP.mult)
                    hn16 = wk.tile([128, DH], f16, tag=f"hn16_{r}",
                                   name=f"hn16_{sfx}", bufs=1)
                    # two half-width ops so transposes of the low half can
                    # start before the high half is normalized
                    for half in range(NH):
                        sl = slice(half * 512, (half + 1) * 512)
                        nc.vector.tensor_scalar(
                            hn16[:, sl], h[r][:, sl], rs_col, nmu[:],
                            op0=OP.mult, op1=OP.add)
                    return hn16

                def transposes(r, hn16, sfx):
                    # paired f16 PE transposes into one [128,256] PSUM tile;
                    # single PSUM->SBUF cast writes the fp8 DoubleRow pair
                    for j in range(KP):
                        tp = pst.tile([128, 256], f16, tag="tp",
                                      name=f"tp_{sfx}_{j}")
                        for i2 in range(2):
                            k = 2 * j + i2
                            nc.tensor.transpose(
                                tp[:, i2 * 128:(i2 + 1) * 128],
                                hn16[:, k * 128:(k + 1) * 128], ident[:])
                        dst = hnT[j][:, :, r * 128:(r + 1) * 128]
                        if j % 2 == 0:
                            nc.vector.tensor_copy(dst, tp[:])
                        else:
                            nc.scalar.copy(dst, tp[:])

                for r in range(RD):
                    hn = normalize(r, mv0[:, 2 * r:2 * r + 1],
                                   rs_p[:, r:r + 1], f"s0_{r}")
                    transposes(r, hn, f"s0_{r}")

                for s in range(steps):
                    last = (s == steps - 1)
                    # mm1: hidT = tanh((W1s' @ hnT)/S1 + b1)  [DoubleRow fp8]
                    for f in range(FD):
                        p1 = ps1p.tile([128, 512], f32, tag="ps1",
                                       name=f"p1_{s}_{f}")
                        for j in range(KP):
                            nc.tensor.matmul(
                                p1[:], w1[j][:, :, f * 128:(f + 1) * 128],
                                hnT[j][:],
                                start=(j == 0), stop=(j == KP - 1),
                                perf_mode=DR)
                        nc.scalar.activation(
                            hidT[f // 2][:, f % 2, :], p1[:], AF.Tanh,
                            bias=b1s[:, f:f + 1], scale=1.0 / S1)

                    if s == 0:
                        # w2/hwt loads deferred past step-0 mm1 so w1 gets
                        # full DMA bandwidth at startup; w2 arrives during
                        # mm1 execution, well before mm2 needs it
                        for j in range(FP):
                            for i2 in range(2):
                                f_ = 2 * j + i2
                                nc.sync.dma_start(
                                    w2[j][:, i2, :],
                                    w2t_d.ap()[f_ * 128:(f_ + 1) * 128, :])
                        for k_ in range(KD):
                            nc.sync.dma_start(
                                hwt[k_][:],
                                hwt_d.ap()[k_ * 128:(k_ + 1) * 128, :])

                    # mm2 per (row-tile, half): p2 = W2s' @ hidT [DoubleRow],
                    # then h += p2; stats/rstd/normalize/transpose interleaved.
                    # transp(r0)/transp(r1) slot between later mm2 groups
                    # (their normalizes have drained by then); r2+r3 stay at
                    # the step end as PE cover while r3's DVE chain drains.
                    mv = stp.tile([128, RD * 2], f32, tag="mv",
                                  name=f"mv_{s}")
                    mvv = mv[:].rearrange("p (r x) -> p r x", x=2)
                    rs = stp.tile([128, RD], f32, tag="rs", name=f"rs_{s}")
                    hns = {}

                    def transp_any(r, sfx):
                        if last:
                            transposes_head(r, hns[r], sfx)
                        else:
                            transposes(r, hns[r], sfx)

                    def transposes_head(r, hn16, sfx):
                        for j in range(KP):
                            tp = pst.tile([128, 256], f16, tag="tp",
                                          name=f"tpf_{sfx}_{j}")
                            for i2 in range(2):
                                k = 2 * j + i2
                                nc.tensor.transpose(
                                    tp[:, i2 * 128:(i2 + 1) * 128],
                                    hn16[:, k * 128:(k + 1) * 128], ident[:])
                            dst = hfT[j][:, :, r * 128:(r + 1) * 128]
                            if j % 2 == 0:
                                nc.vector.tensor_copy(dst, tp[:])
                            else:
                                nc.scalar.copy(dst, tp[:])

                    for r in range(RD):
                        st6 = stp.tile([128, 12], f32, tag="st6",
                                       name=f"st6_{s}_{r}")
                        for half in range(NH):
                            sl = slice(half * 512, (half + 1) * 512)
                            p2 = ps2p.tile([128, 512], f32, tag="ps2",
                                           name=f"p2_{s}_{r}_{half}")
                            # xeg seeded into PSUM via identity matmul (PE is
                            # cheaper here than an extra DVE pass: the DVE is
                            # the step-boundary critical path)
                            nc.tensor.matmul(
                                p2[:], ident[:], xeg[r][:, sl],
                                start=True, stop=False)
                            for j in range(FP):
                                nc.tensor.matmul(
                                    p2[:],
                                    hidT[j][:, :, r * 128:(r + 1) * 128],
                                    w2[j][:, :, sl],
                                    start=False, stop=(j == FP - 1),
                                    perf_mode=DR)
                            nc.vector.scalar_tensor_tensor(
                                h[r][:, sl], h[r][:, sl], 1.0 - GAMMA, p2[:],
                                op0=OP.mult, op1=OP.add)
                            if not last:
                                nc.vector.bn_stats(
                                    st6[:, half * 6:(half + 1) * 6],
                                    h[r][:, sl])
                            if half == 0 and r >= 2:
                                # interleaved transposes of row r-2 (normalize
                                # long since drained; PE slots them between
                                # mm2 groups, casts go behind this row's stats)
                                transp_any(r - 2, f"{s}_{r - 2}")
                        if last:
                            # head prep inline: cast final h to fp16 so its
                            # transposes overlap the remaining matmul2 groups
                            hc16 = wk.tile([128, DH], f16, tag=f"hn16_{r}",
                                           name=f"hc16_{r}", bufs=1)
                            nc.vector.tensor_copy(hc16[:], h[r][:])
                            hns[r] = hc16
                            continue
                        nc.vector.bn_aggr(mvv[:, r], st6[:])
                        rstd_newton(rs[:, r:r + 1], rs_p[:, r:r + 1],
                                    mv[:, 2 * r + 1:2 * r + 2], f"{s}_{r}")
                        hns[r] = normalize(r, mv[:, 2 * r:2 * r + 1],
                                           rs[:, r:r + 1], f"{s}_{r}")
                    for r in (RD - 2, RD - 1):
                        transp_any(r, f"{s}_{r}")
                    rs_p = rs

                # ---- head: outT = (head_w/SH) @ h.T, out = po/SH' + hb ----
                for ot in range(OD):
                    po = ps1p.tile([128, 512], f32, tag="ps1", name=f"po_{ot}")
                    for k in range(KD):
                        nc.tensor.matmul(
                            po[:], hwt[k][:, ot * 128:(ot + 1) * 128],
                            hfT[k // 2][:, k % 2, :],
                            start=(k == 0), stop=(k == KD - 1))
                    osb = wk.tile([128, 512], f32, tag="osb",
                                  name=f"osb_{ot}", bufs=1)
                    nc.scalar.activation(
                        osb[:], po[:], AF.Identity, bias=hbs[:, ot:ot + 1],
                        scale=SH / S2)
                    nc.sync.dma_start(
                        outT_d.ap()[ot * 128:(ot + 1) * 128, :], osb[:])

    nc.compile()
    return nc


def _get_compiled(steps: int):
    key = ("prog", steps)
    if key not in _CACHE:
        from concourse.bass_interp import get_hw_module
        nc = _build_program(steps)
        nc.m = get_hw_module(nc.m)
        _CACHE[key] = nc
    return _CACHE[key]


def _spectral_sigma(W: np.ndarray) -> float:
    W = W.astype(np.float64)
    v = np.full(W.shape[1], 1.0 / np.sqrt(W.shape[1]))
    u = W @ v
    u = u / (np.linalg.norm(u) + 1e-12)
    for _ in range(15):
        u = W @ v
        u = u / (np.linalg.norm(u) + 1e-12)
        v = W.T @ u
        v = v / (np.linalg.norm(v) + 1e-12)
    return float(u @ (W @ v))


def _q8(x: np.ndarray) -> np.ndarray:
    import ml_dtypes
    return np.ascontiguousarray(
        np.clip(x, -240.0, 240.0).astype(ml_dtypes.float8_e4m3))


def _prep_host(inputs: dict) -> tuple[dict, list]:
    f = {k: np.asarray(v, dtype=np.float32) for k, v in inputs.items()}
    x, ew, eb = f["x"], f["embed_w"], f["embed_b"]
    W1, b1, W2, b2 = f["W1"], f["b1"], f["W2"], f["b2"]
    ln_g, ln_b = f["ln_g"], f["ln_b"]
    hw_, hb = f["head_w"], f["head_b"]

    s1 = _spectral_sigma(W1)
    s2 = _spectral_sigma(W2)
    W1n = (W1.astype(np.float64) / s1)
    W2n = (W2.astype(np.float64) / s2)
    # fold ln gain into W1, ln bias into b1
    W1eff = W1n * ln_g.astype(np.float64)[None, :]
    b1eff = (b1.astype(np.float64) + W1n @ ln_b.astype(np.float64))
    W2eff = GAMMA * W2n

    shared = {
        "ewT": np.ascontiguousarray(S2 * ew.T).astype(np.float16),
        "w1t": _q8(np.ascontiguousarray(S1 * W1eff.T)),
        "b1e": b1eff.astype(np.float32).reshape(DFF, 1),
        "w2t": _q8(np.ascontiguousarray(S2 * W2eff.T)),
        "vb1": (S2 * eb).reshape(1, DH).astype(np.float32),
        "vb2": (S2 * GAMMA * b2).reshape(1, DH).astype(np.float32),
        "hwt": np.ascontiguousarray(hw_.T / SH).astype(np.float16),
        "hb": hb.reshape(DOUT, 1).astype(np.float32),
    }
    in_maps = []
    for c in range(N_CORES):
        shard = x[c * R:(c + 1) * R, :]
        m = dict(shared)
        m["xT"] = np.ascontiguousarray(shard.T).astype(np.float16)
        in_maps.append(m)
    return shared, in_maps


def kernel(**inputs) -> np.ndarray:
    from concourse import bass_utils
    nc = _get_compiled(STEPS)
    _, in_maps = _prep_host(inputs)
    res = None
    for attempt in range(3):
        try:
            res = bass_utils.run_bass_kernel_spmd(
                nc, in_maps, core_ids=list(range(N_CORES)))
            break
        except Exception:
            # transient NRT_EXEC_UNIT_UNRECOVERABLE device wedges clear on
            # retry
            if attempt == 2:
                raise
    out = np.empty((B, DOUT), np.float32)
    for c in range(N_CORES):
        out[c * R:(c + 1) * R, :] = res.results[c]["outT"].T
    return out


if __name__ == "__main__":
    rng = np.random.default_rng(0)
    demo = {
        "x": rng.standard_normal((B, DIN)).astype(np.float32),
        "embed_w": (rng.standard_normal((DH, DIN)) * 0.02).astype(np.float32),
        "embed_b": np.zeros(DH, np.float32),
        "W1": (rng.standard_normal((DFF, DH)) * 0.02).astype(np.float32),
        "b1": np.zeros(DFF, np.float32),
        "W2": (rng.standard_normal((DH, DFF)) * 0.02).astype(np.float32),
        "b2": np.zeros(DH, np.float32),
        "ln_g": np.ones(DH, np.float32),
        "ln_b": np.zeros(DH, np.float32),
        "head_w": (rng.standard_normal((DOUT, DH)) * 0.02).astype(np.float32),
        "head_b": np.zeros(DOUT, np.float32),
    }
    out = kernel(**demo)
    print("out", out.shape, out.dtype, float(np.abs(out).max()))


# revision 12
# speedup vs baseline: 2.1153x; 1.0204x over previous
"""Bass/Tile TRN2 kernel for nn_DimensionScaledEqProp.

Data-parallel over batch: x rows sharded across 8 NeuronCores, weights
replicated. Per-core state (h) stays resident in SBUF across the 30
sequential steps.

fp8(e4m3) DoubleRow matmuls for the two big GEMMs (2 MACs/cell/cycle),
fp32 state/accumulation. h is carried scaled by S2=256 so the fp8 weight
scaling folds away: LN is scale-invariant (eps scaled to match), and the
head weights absorb 1/S2.

Self-contained: hardcodes shapes; host side does sharding, spectral-norm
sigma (tiny: 60 matvecs), weight folding/quantization, and the final
gather/transpose.
"""
import sys
import numpy as np

for _p in ("/opt/trn_rl_repo", "/root/.axon_site/_ro/trn_rl_repo"):
    if _p not in sys.path:
        sys.path.append(_p)

B, DIN, DH, DOUT = 4096, 512, 1024, 256
DFF = 4 * DH
STEPS = 30
N_CORES = 8
R = B // N_CORES  # rows per core = 512
GAMMA = 0.5 * min(1.0, float(np.sqrt(64.0 / DIN)))
LN_EPS = 1e-5

KD = DH // 128    # 8  k-tiles over DH
FD = DFF // 128   # 32 f-tiles over DFF
RD = R // 128     # 4  row-tiles per core
ID = DIN // 128   # 4  k-tiles over DIN
OD = DOUT // 128  # 2  out-tiles over DOUT
NH = DH // 512    # 2  psum halves over DH

S1 = 64.0         # fp8 scale on W1eff (undone by tanh's activation scale)
S2 = 256.0        # fp8 scale on W2eff, carried inside h
SH = 16.0         # partial head-weight scale (rest via activation scale)
EPS_DEV = LN_EPS * S2 * S2

KP = KD // 2      # 4  DoubleRow k-pairs for mm1
FP = FD // 2      # 16 DoubleRow f-pairs for mm2

_CACHE = {}


def _build_program(steps: int):
    import concourse.bass as bass
    import concourse.bacc as bacc
    import concourse.mybir as mybir
    from concourse import tile, masks

    f8 = mybir.dt.float8e4
    f16 = mybir.dt.float16
    f32 = mybir.dt.float32
    AF = mybir.ActivationFunctionType
    OP = mybir.AluOpType
    DR = mybir.MatmulPerfMode.DoubleRow

    nc = bacc.Bacc("TRN2", target_bir_lowering=False, debug=False,
                   enable_asserts=True, num_devices=N_CORES)

    xT_d = nc.dram_tensor("xT", [DIN, R], f16, kind="ExternalInput")
    ewT_d = nc.dram_tensor("ewT", [DIN, DH], f16, kind="ExternalInput")
    w1t_d = nc.dram_tensor("w1t", [DH, DFF], f8, kind="ExternalInput")
    b1e_d = nc.dram_tensor("b1e", [DFF, 1], f32, kind="ExternalInput")
    w2t_d = nc.dram_tensor("w2t", [DFF, DH], f8, kind="ExternalInput")
    vb1_d = nc.dram_tensor("vb1", [1, DH], f32, kind="ExternalInput")
    vb2_d = nc.dram_tensor("vb2", [1, DH], f32, kind="ExternalInput")
    hwt_d = nc.dram_tensor("hwt", [DH, DOUT], f16, kind="ExternalInput")
    hb_d = nc.dram_tensor("hb", [DOUT, 1], f32, kind="ExternalInput")
    outT_d = nc.dram_tensor("outT", [DOUT, R], f32, kind="ExternalOutput")

    with tile.TileContext(nc) as tc:
        with (
            tc.tile_pool(name="wp", bufs=1) as wp,
            tc.tile_pool(name="sp", bufs=1) as sp,
            tc.tile_pool(name="wk", bufs=2) as wk,
            tc.tile_pool(name="stp", bufs=2) as stp,
            tc.tile_pool(name="pst", bufs=4, space="PSUM") as pst,
            tc.tile_pool(name="ps1", bufs=2, space="PSUM") as ps1p,
            tc.tile_pool(name="ps2", bufs=2, space="PSUM") as ps2p,
        ):
            # ---- persistent weights / constants ----
            # DoubleRow pair layout: [128, 2, free]; dim1 = second k-subtile
            w1 = [wp.tile([128, 2, DFF], f8, name=f"w1_{j}") for j in range(KP)]
            w2 = [wp.tile([128, 2, DH], f8, name=f"w2_{j}") for j in range(FP)]
            hwt = [wp.tile([128, DOUT], f16, name=f"hwt_{k}") for k in range(KD)]
            b1s = wp.tile([128, FD], f32, name="b1s")
            hbs = wp.tile([128, OD], f32, name="hbs")
            ident = wp.tile([128, 128], f16, name="ident")

            # ---- persistent state ----
            h = [sp.tile([128, DH], f32, name=f"h_{r}") for r in range(RD)]
            xeg = [sp.tile([128, DH], f16, name=f"xeg_{r}") for r in range(RD)]
            hnT = [sp.tile([128, 2, R], f8, name=f"hnT_{j}") for j in range(KP)]
            hfT = [sp.tile([128, 2, R], f16, name=f"hfT_{j}")
                   for j in range(KP)]

            masks.make_identity(nc, ident[:])

            # ---- embed (transient pool, released before the step loop) ----
            with tc.tile_pool(name="ep", bufs=1) as ep:
                xts = [ep.tile([128, R], f16, name=f"xts_{i}")
                       for i in range(ID)]
                ewt = [ep.tile([128, DH], f16, name=f"ewt_{i}")
                       for i in range(ID)]
                bc1 = ep.tile([128, DH], f32, name="bc1")
                bc2 = ep.tile([128, DH], f32, name="bc2")
                for i in range(ID):
                    nc.sync.dma_start(
                        xts[i][:], xT_d.ap()[i * 128:(i + 1) * 128, :])
                    nc.sync.dma_start(
                        ewt[i][:], ewT_d.ap()[i * 128:(i + 1) * 128, :])
                nc.sync.dma_start(bc1[0:1, :], vb1_d.ap())
                nc.sync.dma_start(bc2[0:1, :], vb2_d.ap())
                nc.gpsimd.partition_broadcast(bc1[:], bc1[0:1, :])
                nc.gpsimd.partition_broadcast(bc2[:], bc2[0:1, :])

                # weight loads AFTER embed inputs: embed matmuls start
                # immediately; w1/w2 stream in behind them
                for j in range(KP):
                    for i2 in range(2):
                        k = 2 * j + i2
                        nc.sync.dma_start(
                            w1[j][:, i2, :],
                            w1t_d.ap()[k * 128:(k + 1) * 128, :])
                nc.sync.dma_start(
                    b1s[:], b1e_d.ap().rearrange("(f p) o -> p (f o)", p=128))
                nc.sync.dma_start(
                    hbs[:], hb_d.ap().rearrange("(t p) o -> p (t o)", p=128))

                # h0 = x @ ewT + embed_b ; xeg = g*h0 + g*S2*b2 (f16)
                for r in range(RD):
                    for half in range(NH):
                        sl = slice(half * 512, (half + 1) * 512)
                        pe = ps1p.tile([128, 512], f32, tag="ps1", name="pe")
                        for i in range(ID):
                            nc.tensor.matmul(
                                pe[:], xts[i][:, r * 128:(r + 1) * 128],
                                ewt[i][:, sl],
                                start=(i == 0), stop=(i == ID - 1))
                        nc.vector.tensor_tensor(
                            h[r][:, sl], pe[:], bc1[:, sl], op=OP.add)
                        nc.vector.scalar_tensor_tensor(
                            xeg[r][:, sl], h[r][:, sl], GAMMA, bc2[:, sl],
                            op0=OP.mult, op1=OP.add)

            # ---- initial LN stats on h0 (ACT sqrt once; hides table load) ----
            mv0 = stp.tile([128, RD * 2], f32, tag="mv", name="mv_init")
            for r in range(RD):
                st6 = stp.tile([128, 12], f32, tag="st6", name=f"st6_i_{r}")
                for c in range(2):
                    nc.vector.bn_stats(
                        st6[:, c * 6:(c + 1) * 6],
                        h[r][:, c * 512:(c + 1) * 512])
                nc.vector.bn_aggr(
                    mv0[:].rearrange("p (r x) -> p r x", x=2)[:, r], st6[:])
            ve0 = stp.tile([128, RD], f32, tag="ve", name="ve_init")
            nc.vector.tensor_scalar(
                ve0[:], mv0[:].rearrange("p (r x) -> p x r", x=2)[:, 1],
                EPS_DEV, None, op0=OP.add)
            rv0 = stp.tile([128, RD], f32, tag="rv", name="rv_init")
            nc.vector.reciprocal(rv0[:], ve0[:])
            rs_prev = stp.tile([128, RD], f32, tag="rs", name="rs_init")
            nc.scalar.activation(rs_prev[:], rv0[:], AF.Sqrt)

            # ---- hidT pool reuses the embed pool space ----
            with tc.tile_pool(name="hp", bufs=1) as hp:
                hidT = [hp.tile([128, 2, R], f8, name=f"hidT_{j}")
                        for j in range(FP)]

                def rstd_newton(y_out, y_seed, var_ap, tag_sfx, n_iter=2):
                    """y_out[128,1] = 1/sqrt(var+eps) via Newton from seed."""
                    hv = stp.tile([128, 1], f32, tag="hv",
                                  name=f"hv_{tag_sfx}")
                    nc.vector.tensor_scalar(
                        hv[:], var_ap, -0.5, -0.5 * EPS_DEV,
                        op0=OP.mult, op1=OP.add)
                    y = y_seed
                    for it in range(n_iter):
                        a = stp.tile([128, 1], f32, tag="nwa",
                                     name=f"nwa_{tag_sfx}_{it}")
                        nc.vector.tensor_tensor(a[:], y, y, op=OP.mult)
                        nc.vector.tensor_scalar(
                            a[:], a[:], hv[:], 1.5, op0=OP.mult, op1=OP.add)
                        if it == n_iter - 1:
                            nc.vector.tensor_tensor(y_out, y, a[:], op=OP.mult)
                        else:
                            yn = stp.tile([128, 1], f32, tag="nwy",
                                          name=f"nwy_{tag_sfx}_{it}")
                            nc.vector.tensor_tensor(yn[:], y, a[:], op=OP.mult)
                            y = yn[:]

                # normalize h0 -> hnT for step 0 (one Newton polish on seed)
                rs_fix = stp.tile([128, RD], f32, tag="rsf", name="rs_fix")
                for r in range(RD):
                    rstd_newton(rs_fix[:, r:r + 1], rs_prev[:, r:r + 1],
                                mv0[:, 2 * r + 1:2 * r + 2], f"i{r}", n_iter=1)
                rs_p = rs_fix

                def normalize(r, mean_ap, rs_col, sfx):
                    nmu = stp.tile([128, 1], f32, tag="nmu", name=f"nmu_{sfx}")
                    nc.vector.scalar_tensor_tensor(
                        nmu[:], mean_ap, -1.0, rs_col,
                        op0=OP.mult, op1=OP.mult)
                    hn16 = wk.tile([128, DH], f16, tag=f"hn16_{r}",
                                   name=f"hn16_{sfx}", bufs=1)
                    # two half-width ops so transposes of the low half can
                    # start before the high half is normalized
                    for half in range(NH):
                        sl = slice(half * 512, (half + 1) * 512)
                        nc.vector.tensor_scalar(
                            hn16[:, sl], h[r][:, sl], rs_col, nmu[:],
                            op0=OP.mult, op1=OP.add)
                    return hn16

                def transposes(r, hn16, sfx):
                    # paired f16 PE transposes into one [128,256] PSUM tile;
                    # single PSUM->SBUF cast writes the fp8 DoubleRow pair
                    for j in range(KP):
                        tp = pst.tile([128, 256], f16, tag="tp",
                                      name=f"tp_{sfx}_{j}")
                        for i2 in range(2):
                            k = 2 * j + i2
                            nc.tensor.transpose(
                                tp[:, i2 * 128:(i2 + 1) * 128],
                                hn16[:, k * 128:(k + 1) * 128], ident[:])
                        dst = hnT[j][:, :, r * 128:(r + 1) * 128]
                        if j % 2 == 0:
                            nc.vector.tensor_copy(dst, tp[:])
                        else:
                            nc.scalar.copy(dst, tp[:])

                for r in range(RD):
                    hn = normalize(r, mv0[:, 2 * r:2 * r + 1],
                                   rs_p[:, r:r + 1], f"s0_{r}")
                    transposes(r, hn, f"s0_{r}")

                for s in range(steps):
                    last = (s == steps - 1)
                    # mm1: hidT = tanh((W1s' @ hnT)/S1 + b1)  [DoubleRow fp8]
                    for f in range(FD):
                        p1 = ps1p.tile([128, 512], f32, tag="ps1",
                                       name=f"p1_{s}_{f}")
                        for j in range(KP):
                            nc.tensor.matmul(
                                p1[:], w1[j][:, :, f * 128:(f + 1) * 128],
                                hnT[j][:],
                                start=(j == 0), stop=(j == KP - 1),
                                perf_mode=DR)
                        nc.scalar.activation(
                            hidT[f // 2][:, f % 2, :], p1[:], AF.Tanh,
                            bias=b1s[:, f:f + 1], scale=1.0 / S1)

                    if s == 0:
                        # w2/hwt loads deferred past step-0 mm1 so w1 gets
                        # full DMA bandwidth at startup; w2 arrives during
                        # mm1 execution, well before mm2 needs it
                        for j in range(FP):
                            for i2 in range(2):
                                f_ = 2 * j + i2
                                nc.sync.dma_start(
                                    w2[j][:, i2, :],
                                    w2t_d.ap()[f_ * 128:(f_ + 1) * 128, :])
                        for k_ in range(KD):
                            nc.sync.dma_start(
                                hwt[k_][:],
                                hwt_d.ap()[k_ * 128:(k_ + 1) * 128, :])

                    # mm2 per (row-tile, half): p2 = W2s' @ hidT [DoubleRow],
                    # then h += p2; stats/rstd/normalize/transpose interleaved.
                    # transp(r0)/transp(r1) slot between later mm2 groups
                    # (their normalizes have drained by then); r2+r3 stay at
                    # the step end as PE cover while r3's DVE chain drains.
                    mv = stp.tile([128, RD * 2], f32, tag="mv",
                                  name=f"mv_{s}")
                    mvv = mv[:].rearrange("p (r x) -> p r x", x=2)
                    rs = stp.tile([128, RD], f32, tag="rs", name=f"rs_{s}")
                    hns = {}

                    # fused 1-iter Newton for rstd once var drift is small
                    # (s>=4): rs = y*(var*s1 + s2), s1/s2 precomputed from the
                    # seed y during mm1 so only 2 small DVE ops sit on the
                    # step-boundary critical path.
                    fused = (s >= 4) and not last
                    if fused:
                        nsc = stp.tile([128, 2 * RD], f32, tag="nsc",
                                       name=f"nsc_{s}")
                        yy = stp.tile([128, RD], f32, tag="nyy",
                                      name=f"nyy_{s}")
                        nc.vector.tensor_tensor(
                            yy[:], rs_p[:], rs_p[:], op=OP.mult)
                        nscv = nsc[:].rearrange("p (x r) -> p x r", x=2)
                        nc.vector.tensor_scalar(
                            nscv[:, 0], yy[:], -0.5, None, op0=OP.mult)
                        nc.vector.tensor_scalar(
                            nscv[:, 1], yy[:], -0.5 * EPS_DEV, 1.5,
                            op0=OP.mult, op1=OP.add)

                    def transp_any(r, sfx):
                        if last:
                            transposes_head(r, hns[r], sfx)
                        else:
                            transposes(r, hns[r], sfx)

                    def transposes_head(r, hn16, sfx):
                        for j in range(KP):
                            tp = pst.tile([128, 256], f16, tag="tp",
                                          name=f"tpf_{sfx}_{j}")
                            for i2 in range(2):
                                k = 2 * j + i2
                                nc.tensor.transpose(
                                    tp[:, i2 * 128:(i2 + 1) * 128],
                                    hn16[:, k * 128:(k + 1) * 128], ident[:])
                            dst = hfT[j][:, :, r * 128:(r + 1) * 128]
                            if j % 2 == 0:
                                nc.vector.tensor_copy(dst, tp[:])
                            else:
                                nc.scalar.copy(dst, tp[:])

                    for r in range(RD):
                        st6 = stp.tile([128, 12], f32, tag="st6",
                                       name=f"st6_{s}_{r}")
                        for half in range(NH):
                            sl = slice(half * 512, (half + 1) * 512)
                            p2 = ps2p.tile([128, 512], f32, tag="ps2",
                                           name=f"p2_{s}_{r}_{half}")
                            # xeg seeded into PSUM via identity matmul (PE is
                            # cheaper here than an extra DVE pass: the DVE is
                            # the step-boundary critical path)
                            nc.tensor.matmul(
                                p2[:], ident[:], xeg[r][:, sl],
                                start=True, stop=False)
                            for j in range(FP):
                                nc.tensor.matmul(
                                    p2[:],
                                    hidT[j][:, :, r * 128:(r + 1) * 128],
                                    w2[j][:, :, sl],
                                    start=False, stop=(j == FP - 1),
                                    perf_mode=DR)
                            nc.vector.scalar_tensor_tensor(
                                h[r][:, sl], h[r][:, sl], 1.0 - GAMMA, p2[:],
                                op0=OP.mult, op1=OP.add)
                            if not last:
                                nc.vector.bn_stats(
                                    st6[:, half * 6:(half + 1) * 6],
                                    h[r][:, sl])
                            if half == 0 and r >= 2:
                                # interleaved transposes of row r-2 (normalize
                                # long since drained; PE slots them between
                                # mm2 groups, casts go behind this row's stats)
                                transp_any(r - 2, f"{s}_{r - 2}")
                        if last:
                            # head prep inline: cast final h to fp16 so its
                            # transposes overlap the remaining matmul2 groups
                            hc16 = wk.tile([128, DH], f16, tag=f"hn16_{r}",
                                           name=f"hc16_{r}", bufs=1)
                            nc.vector.tensor_copy(hc16[:], h[r][:])
                            hns[r] = hc16
                            continue
                        nc.vector.bn_aggr(mvv[:, r], st6[:])
                        if fused:
                            a = stp.tile([128, 1], f32, tag="nwa",
                                         name=f"nfa_{s}_{r}")
                            nc.vector.tensor_scalar(
                                a[:], mv[:, 2 * r + 1:2 * r + 2],
                                nscv[:, 0, r:r + 1], nscv[:, 1, r:r + 1],
                                op0=OP.mult, op1=OP.add)
                            nc.vector.tensor_tensor(
                                rs[:, r:r + 1], rs_p[:, r:r + 1], a[:],
                                op=OP.mult)
                        else:
                            rstd_newton(rs[:, r:r + 1], rs_p[:, r:r + 1],
                                        mv[:, 2 * r + 1:2 * r + 2],
                                        f"{s}_{r}")
                        hns[r] = normalize(r, mv[:, 2 * r:2 * r + 1],
                                           rs[:, r:r + 1], f"{s}_{r}")
                    for r in (RD - 2, RD - 1):
                        transp_any(r, f"{s}_{r}")
                    rs_p = rs

                # ---- head: outT = (head_w/SH) @ h.T, out = po/SH' + hb ----
                for ot in range(OD):
                    po = ps1p.tile([128, 512], f32, tag="ps1", name=f"po_{ot}")
                    for k in range(KD):
                        nc.tensor.matmul(
                            po[:], hwt[k][:, ot * 128:(ot + 1) * 128],
                            hfT[k // 2][:, k % 2, :],
                            start=(k == 0), stop=(k == KD - 1))
                    osb = wk.tile([128, 512], f32, tag="osb",
                                  name=f"osb_{ot}", bufs=1)
                    nc.scalar.activation(
                        osb[:], po[:], AF.Identity, bias=hbs[:, ot:ot + 1],
                        scale=SH / S2)
                    nc.sync.dma_start(
                        outT_d.ap()[ot * 128:(ot + 1) * 128, :], osb[:])

    nc.compile()
    return nc


def _get_compiled(steps: int):
    key = ("prog", steps)
    if key not in _CACHE:
        from concourse.bass_interp import get_hw_module
        nc = _build_program(steps)
        nc.m = get_hw_module(nc.m)
        _CACHE[key] = nc
    return _CACHE[key]


def _spectral_sigma(W: np.ndarray) -> float:
    W = W.astype(np.float64)
    v = np.full(W.shape[1], 1.0 / np.sqrt(W.shape[1]))
    u = W @ v
    u = u / (np.linalg.norm(u) + 1e-12)
    for _ in range(15):
        u = W @ v
        u = u / (np.linalg.norm(u) + 1e-12)
        v = W.T @ u
        v = v / (np.linalg.norm(v) + 1e-12)
    return float(u @ (W @ v))


def _q8(x: np.ndarray) -> np.ndarray:
    import ml_dtypes
    return np.ascontiguousarray(
        np.clip(x, -240.0, 240.0).astype(ml_dtypes.float8_e4m3))


def _prep_host(inputs: dict) -> tuple[dict, list]:
    f = {k: np.asarray(v, dtype=np.float32) for k, v in inputs.items()}
    x, ew, eb = f["x"], f["embed_w"], f["embed_b"]
    W1, b1, W2, b2 = f["W1"], f["b1"], f["W2"], f["b2"]
    ln_g, ln_b = f["ln_g"], f["ln_b"]
    hw_, hb = f["head_w"], f["head_b"]

    s1 = _spectral_sigma(W1)
    s2 = _spectral_sigma(W2)
    W1n = (W1.astype(np.float64) / s1)
    W2n = (W2.astype(np.float64) / s2)
    # fold ln gain into W1, ln bias into b1
    W1eff = W1n * ln_g.astype(np.float64)[None, :]
    b1eff = (b1.astype(np.float64) + W1n @ ln_b.astype(np.float64))
    W2eff = GAMMA * W2n

    shared = {
        "ewT": np.ascontiguousarray(S2 * ew.T).astype(np.float16),
        "w1t": _q8(np.ascontiguousarray(S1 * W1eff.T)),
        "b1e": b1eff.astype(np.float32).reshape(DFF, 1),
        "w2t": _q8(np.ascontiguousarray(S2 * W2eff.T)),
        "vb1": (S2 * eb).reshape(1, DH).astype(np.float32),
        "vb2": (S2 * GAMMA * b2).reshape(1, DH).astype(np.float32),
        "hwt": np.ascontiguousarray(hw_.T / SH).astype(np.float16),
        "hb": hb.reshape(DOUT, 1).astype(np.float32),
    }
    in_maps = []
    for c in range(N_CORES):
        shard = x[c * R:(c + 1) * R, :]
        m = dict(shared)
        m["xT"] = np.ascontiguousarray(shard.T).astype(np.float16)
        in_maps.append(m)
    return shared, in_maps


def kernel(**inputs) -> np.ndarray:
    from concourse import bass_utils
    nc = _get_compiled(STEPS)
    _, in_maps = _prep_host(inputs)
    res = None
    for attempt in range(3):
        try:
            res = bass_utils.run_bass_kernel_spmd(
                nc, in_maps, core_ids=list(range(N_CORES)))
            break
        except Exception:
            # transient NRT_EXEC_UNIT_UNRECOVERABLE device wedges clear on
            # retry
            if attempt == 2:
                raise
    out = np.empty((B, DOUT), np.float32)
    for c in range(N_CORES):
        out[c * R:(c + 1) * R, :] = res.results[c]["outT"].T
    return out


if __name__ == "__main__":
    rng = np.random.default_rng(0)
    demo = {
        "x": rng.standard_normal((B, DIN)).astype(np.float32),
        "embed_w": (rng.standard_normal((DH, DIN)) * 0.02).astype(np.float32),
        "embed_b": np.zeros(DH, np.float32),
        "W1": (rng.standard_normal((DFF, DH)) * 0.02).astype(np.float32),
        "b1": np.zeros(DFF, np.float32),
        "W2": (rng.standard_normal((DH, DFF)) * 0.02).astype(np.float32),
        "b2": np.zeros(DH, np.float32),
        "ln_g": np.ones(DH, np.float32),
        "ln_b": np.zeros(DH, np.float32),
        "head_w": (rng.standard_normal((DOUT, DH)) * 0.02).astype(np.float32),
        "head_b": np.zeros(DOUT, np.float32),
    }
    out = kernel(**demo)
    print("out", out.shape, out.dtype, float(np.abs(out).max()))


# revision 16
# speedup vs baseline: 2.1168x; 1.0007x over previous
"""Bass/Tile TRN2 kernel for nn_DimensionScaledEqProp.

Data-parallel over batch: x rows sharded across 8 NeuronCores, weights
replicated. Per-core state (h) stays resident in SBUF across the 30
sequential steps.

fp8(e4m3) DoubleRow matmuls for the two big GEMMs (2 MACs/cell/cycle),
fp32 state/accumulation. h is carried scaled by S2=256 so the fp8 weight
scaling folds away: LN is scale-invariant (eps scaled to match), and the
head weights absorb 1/S2.

Self-contained: hardcodes shapes; host side does sharding, spectral-norm
sigma (tiny: 60 matvecs), weight folding/quantization, and the final
gather/transpose.
"""
import sys
import numpy as np

for _p in ("/opt/trn_rl_repo", "/root/.axon_site/_ro/trn_rl_repo"):
    if _p not in sys.path:
        sys.path.append(_p)

B, DIN, DH, DOUT = 4096, 512, 1024, 256
DFF = 4 * DH
STEPS = 30
N_CORES = 8
R = B // N_CORES  # rows per core = 512
GAMMA = 0.5 * min(1.0, float(np.sqrt(64.0 / DIN)))
LN_EPS = 1e-5

KD = DH // 128    # 8  k-tiles over DH
FD = DFF // 128   # 32 f-tiles over DFF
RD = R // 128     # 4  row-tiles per core
ID = DIN // 128   # 4  k-tiles over DIN
OD = DOUT // 128  # 2  out-tiles over DOUT
NH = DH // 512    # 2  psum halves over DH

S1 = 64.0         # fp8 scale on W1eff (undone by tanh's activation scale)
S2 = 256.0        # fp8 scale on W2eff, carried inside h
SH = 16.0         # partial head-weight scale (rest via activation scale)
EPS_DEV = LN_EPS * S2 * S2

KP = KD // 2      # 4  DoubleRow k-pairs for mm1
FP = FD // 2      # 16 DoubleRow f-pairs for mm2

_CACHE = {}


def _build_program(steps: int):
    import concourse.bass as bass
    import concourse.bacc as bacc
    import concourse.mybir as mybir
    from concourse import tile, masks

    f8 = mybir.dt.float8e4
    f16 = mybir.dt.float16
    f32 = mybir.dt.float32
    AF = mybir.ActivationFunctionType
    OP = mybir.AluOpType
    DR = mybir.MatmulPerfMode.DoubleRow

    nc = bacc.Bacc("TRN2", target_bir_lowering=False, debug=False,
                   enable_asserts=True, num_devices=N_CORES)

    xT_d = nc.dram_tensor("xT", [DIN, R], f16, kind="ExternalInput")
    ewT_d = nc.dram_tensor("ewT", [DIN, DH], f16, kind="ExternalInput")
    w1t_d = nc.dram_tensor("w1t", [DH, DFF], f8, kind="ExternalInput")
    b1e_d = nc.dram_tensor("b1e", [DFF, 1], f32, kind="ExternalInput")
    w2t_d = nc.dram_tensor("w2t", [DFF, DH], f8, kind="ExternalInput")
    vb1_d = nc.dram_tensor("vb1", [1, DH], f16, kind="ExternalInput")
    vb2_d = nc.dram_tensor("vb2", [1, DH], f32, kind="ExternalInput")
    hwt_d = nc.dram_tensor("hwt", [DH, DOUT], f16, kind="ExternalInput")
    hb_d = nc.dram_tensor("hb", [DOUT, 1], f32, kind="ExternalInput")
    outT_d = nc.dram_tensor("outT", [DOUT, R], f32, kind="ExternalOutput")

    with tile.TileContext(nc) as tc:
        with (
            tc.tile_pool(name="wp", bufs=1) as wp,
            tc.tile_pool(name="sp", bufs=1) as sp,
            tc.tile_pool(name="wk", bufs=2) as wk,
            tc.tile_pool(name="stp", bufs=2) as stp,
            tc.tile_pool(name="pst", bufs=4, space="PSUM") as pst,
            tc.tile_pool(name="ps1", bufs=2, space="PSUM") as ps1p,
            tc.tile_pool(name="ps2", bufs=2, space="PSUM") as ps2p,
        ):
            # ---- persistent weights / constants ----
            # DoubleRow pair layout: [128, 2, free]; dim1 = second k-subtile
            w1 = [wp.tile([128, 2, DFF], f8, name=f"w1_{j}") for j in range(KP)]
            w2 = [wp.tile([128, 2, DH], f8, name=f"w2_{j}") for j in range(FP)]
            hwt = [wp.tile([128, DOUT], f16, name=f"hwt_{k}") for k in range(KD)]
            b1s = wp.tile([128, FD], f32, name="b1s")
            hbs = wp.tile([128, OD], f32, name="hbs")
            ident = wp.tile([128, 128], f16, name="ident")

            # ---- persistent state ----
            h = [sp.tile([128, DH], f32, name=f"h_{r}") for r in range(RD)]
            xeg = [sp.tile([128, DH], f16, name=f"xeg_{r}") for r in range(RD)]
            hnT = [sp.tile([128, 2, R], f8, name=f"hnT_{j}") for j in range(KP)]
            hfT = [sp.tile([128, 2, R], f16, name=f"hfT_{j}")
                   for j in range(KP)]

            masks.make_identity(nc, ident[:])

            # ---- embed (transient pool, released before the step loop) ----
            mv0 = stp.tile([128, RD * 2], f32, tag="mv", name="mv_init")
            with tc.tile_pool(name="ep", bufs=1) as ep:
                xts = [ep.tile([128, R], f16, name=f"xts_{i}")
                       for i in range(ID)]
                ewt = [ep.tile([128, DH], f16, name=f"ewt_{i}")
                       for i in range(ID)]
                bc1 = ep.tile([128, DH], f16, name="bc1")
                bc2 = ep.tile([128, DH], f32, name="bc2")
                for i in range(ID):
                    nc.sync.dma_start(
                        xts[i][:], xT_d.ap()[i * 128:(i + 1) * 128, :])
                    nc.sync.dma_start(
                        ewt[i][:], ewT_d.ap()[i * 128:(i + 1) * 128, :])
                nc.sync.dma_start(bc1[0:1, :], vb1_d.ap())
                nc.sync.dma_start(bc2[0:1, :], vb2_d.ap())
                nc.gpsimd.partition_broadcast(bc1[:], bc1[0:1, :])
                nc.gpsimd.partition_broadcast(bc2[:], bc2[0:1, :])

                # weight loads AFTER embed inputs: embed matmuls start
                # immediately; w1/w2 stream in behind them
                for j in range(KP):
                    for i2 in range(2):
                        k = 2 * j + i2
                        nc.sync.dma_start(
                            w1[j][:, i2, :],
                            w1t_d.ap()[k * 128:(k + 1) * 128, :])
                nc.sync.dma_start(
                    b1s[:], b1e_d.ap().rearrange("(f p) o -> p (f o)", p=128))
                nc.sync.dma_start(
                    hbs[:], hb_d.ap().rearrange("(t p) o -> p (t o)", p=128))

                # h0 = x @ ewT + embed_b (bias seeded into PSUM on the PE,
                # PSUM->SBUF copy on ACT, stats on DVE: three-engine pipeline)
                for r in range(RD):
                    for half in range(NH):
                        sl = slice(half * 512, (half + 1) * 512)
                        pe = ps1p.tile([128, 512], f32, tag="ps1", name="pe")
                        nc.tensor.matmul(
                            pe[:], ident[:], bc1[:, sl],
                            start=True, stop=False)
                        for i in range(ID):
                            nc.tensor.matmul(
                                pe[:], xts[i][:, r * 128:(r + 1) * 128],
                                ewt[i][:, sl],
                                start=False, stop=(i == ID - 1))
                        nc.scalar.copy(h[r][:, sl], pe[:])
                    st6 = stp.tile([128, 12], f32, tag="st6",
                                   name=f"st6_i_{r}")
                    for c in range(2):
                        nc.vector.bn_stats(
                            st6[:, c * 6:(c + 1) * 6],
                            h[r][:, c * 512:(c + 1) * 512])
                    nc.vector.bn_aggr(
                        mv0[:].rearrange("p (r x) -> p r x", x=2)[:, r],
                        st6[:])
                # xeg = g*h0 + g*S2*b2 (f16) — off the init-LN critical path
                for r in range(RD):
                    for half in range(NH):
                        sl = slice(half * 512, (half + 1) * 512)
                        nc.vector.scalar_tensor_tensor(
                            xeg[r][:, sl], h[r][:, sl], GAMMA, bc2[:, sl],
                            op0=OP.mult, op1=OP.add)

            # ---- initial rstd (ACT sqrt once; hides table load) ----
            ve0 = stp.tile([128, RD], f32, tag="ve", name="ve_init")
            nc.vector.tensor_scalar(
                ve0[:], mv0[:].rearrange("p (r x) -> p x r", x=2)[:, 1],
                EPS_DEV, None, op0=OP.add)
            rv0 = stp.tile([128, RD], f32, tag="rv", name="rv_init")
            nc.vector.reciprocal(rv0[:], ve0[:])
            rs_prev = stp.tile([128, RD], f32, tag="rs", name="rs_init")
            nc.scalar.activation(rs_prev[:], rv0[:], AF.Sqrt)

            # ---- hidT pool reuses the embed pool space ----
            with tc.tile_pool(name="hp", bufs=1) as hp:
                hidT = [hp.tile([128, 2, R], f8, name=f"hidT_{j}")
                        for j in range(FP)]

                def rstd_newton(y_out, y_seed, var_ap, tag_sfx, n_iter=2):
                    """y_out[128,1] = 1/sqrt(var+eps) via Newton from seed."""
                    hv = stp.tile([128, 1], f32, tag="hv",
                                  name=f"hv_{tag_sfx}")
                    nc.vector.tensor_scalar(
                        hv[:], var_ap, -0.5, -0.5 * EPS_DEV,
                        op0=OP.mult, op1=OP.add)
                    y = y_seed
                    for it in range(n_iter):
                        a = stp.tile([128, 1], f32, tag="nwa",
                                     name=f"nwa_{tag_sfx}_{it}")
                        nc.vector.tensor_tensor(a[:], y, y, op=OP.mult)
                        nc.vector.tensor_scalar(
                            a[:], a[:], hv[:], 1.5, op0=OP.mult, op1=OP.add)
                        if it == n_iter - 1:
                            nc.vector.tensor_tensor(y_out, y, a[:], op=OP.mult)
                        else:
                            yn = stp.tile([128, 1], f32, tag="nwy",
                                          name=f"nwy_{tag_sfx}_{it}")
                            nc.vector.tensor_tensor(yn[:], y, a[:], op=OP.mult)
                            y = yn[:]

                # normalize h0 -> hnT for step 0 (one Newton polish on seed)
                rs_fix = stp.tile([128, RD], f32, tag="rsf", name="rs_fix")
                for r in range(RD):
                    rstd_newton(rs_fix[:, r:r + 1], rs_prev[:, r:r + 1],
                                mv0[:, 2 * r + 1:2 * r + 2], f"i{r}", n_iter=1)
                rs_p = rs_fix

                def normalize(r, mean_ap, rs_col, sfx):
                    nmu = stp.tile([128, 1], f32, tag="nmu", name=f"nmu_{sfx}")
                    nc.vector.scalar_tensor_tensor(
                        nmu[:], mean_ap, -1.0, rs_col,
                        op0=OP.mult, op1=OP.mult)
                    hn16 = wk.tile([128, DH], f16, tag=f"hn16_{r}",
                                   name=f"hn16_{sfx}", bufs=1)
                    # two half-width ops so transposes of the low half can
                    # start before the high half is normalized
                    for half in range(NH):
                        sl = slice(half * 512, (half + 1) * 512)
                        nc.vector.tensor_scalar(
                            hn16[:, sl], h[r][:, sl], rs_col, nmu[:],
                            op0=OP.mult, op1=OP.add)
                    return hn16

                def transposes(r, hn16, sfx):
                    # paired f16 PE transposes into one [128,256] PSUM tile;
                    # single PSUM->SBUF cast writes the fp8 DoubleRow pair.
                    # rows 2/3 cast on ACT only: their casts land at the step
                    # boundary where the DVE queue is the critical path
                    for j in range(KP):
                        tp = pst.tile([128, 256], f16, tag="tp",
                                      name=f"tp_{sfx}_{j}")
                        for i2 in range(2):
                            k = 2 * j + i2
                            nc.tensor.transpose(
                                tp[:, i2 * 128:(i2 + 1) * 128],
                                hn16[:, k * 128:(k + 1) * 128], ident[:])
                        dst = hnT[j][:, :, r * 128:(r + 1) * 128]
                        if r < 2 and j % 2 == 0:
                            nc.vector.tensor_copy(dst, tp[:])
                        else:
                            nc.scalar.copy(dst, tp[:])

                for r in range(RD):
                    hn = normalize(r, mv0[:, 2 * r:2 * r + 1],
                                   rs_p[:, r:r + 1], f"s0_{r}")
                    transposes(r, hn, f"s0_{r}")

                for s in range(steps):
                    last = (s == steps - 1)
                    # mm1: hidT = tanh((W1s' @ hnT)/S1 + b1)  [DoubleRow fp8]
                    for f in range(FD):
                        p1 = ps1p.tile([128, 512], f32, tag="ps1",
                                       name=f"p1_{s}_{f}")
                        for j in range(KP):
                            nc.tensor.matmul(
                                p1[:], w1[j][:, :, f * 128:(f + 1) * 128],
                                hnT[j][:],
                                start=(j == 0), stop=(j == KP - 1),
                                perf_mode=DR)
                        nc.scalar.activation(
                            hidT[f // 2][:, f % 2, :], p1[:], AF.Tanh,
                            bias=b1s[:, f:f + 1], scale=1.0 / S1)

                    if s == 0:
                        # w2/hwt loads deferred past step-0 mm1 so w1 gets
                        # full DMA bandwidth at startup; w2 arrives during
                        # mm1 execution, well before mm2 needs it
                        for j in range(FP):
                            for i2 in range(2):
                                f_ = 2 * j + i2
                                nc.sync.dma_start(
                                    w2[j][:, i2, :],
                                    w2t_d.ap()[f_ * 128:(f_ + 1) * 128, :])
                        for k_ in range(KD):
                            nc.sync.dma_start(
                                hwt[k_][:],
                                hwt_d.ap()[k_ * 128:(k_ + 1) * 128, :])

                    # mm2 per (row-tile, half): p2 = W2s' @ hidT [DoubleRow],
                    # then h += p2; stats/rstd/normalize/transpose interleaved.
                    # transp(r0)/transp(r1) slot between later mm2 groups
                    # (their normalizes have drained by then); r2+r3 stay at
                    # the step end as PE cover while r3's DVE chain drains.
                    mv = stp.tile([128, RD * 2], f32, tag="mv",
                                  name=f"mv_{s}")
                    mvv = mv[:].rearrange("p (r x) -> p r x", x=2)
                    rs = stp.tile([128, RD], f32, tag="rs", name=f"rs_{s}")
                    hns = {}

                    # fused 1-iter Newton for rstd once var drift is small
                    # (s>=4): rs = y*(var*s1 + s2), s1/s2 precomputed from the
                    # seed y during mm1 so only 2 small DVE ops sit on the
                    # step-boundary critical path.
                    fused = (s >= 4) and not last
                    if fused:
                        nsc = stp.tile([128, 2 * RD], f32, tag="nsc",
                                       name=f"nsc_{s}")
                        yy = stp.tile([128, RD], f32, tag="nyy",
                                      name=f"nyy_{s}")
                        nc.vector.tensor_tensor(
                            yy[:], rs_p[:], rs_p[:], op=OP.mult)
                        nscv = nsc[:].rearrange("p (x r) -> p x r", x=2)
                        nc.vector.tensor_scalar(
                            nscv[:, 0], yy[:], -0.5, None, op0=OP.mult)
                        nc.vector.tensor_scalar(
                            nscv[:, 1], yy[:], -0.5 * EPS_DEV, 1.5,
                            op0=OP.mult, op1=OP.add)

                    def transp_any(r, sfx):
                        if last:
                            transposes_head(r, hns[r], sfx)
                        else:
                            transposes(r, hns[r], sfx)

                    def transposes_head(r, hn16, sfx):
                        for j in range(KP):
                            tp = pst.tile([128, 256], f16, tag="tp",
                                          name=f"tpf_{sfx}_{j}")
                            for i2 in range(2):
                                k = 2 * j + i2
                                nc.tensor.transpose(
                                    tp[:, i2 * 128:(i2 + 1) * 128],
                                    hn16[:, k * 128:(k + 1) * 128], ident[:])
                            dst = hfT[j][:, :, r * 128:(r + 1) * 128]
                            if j % 2 == 0:
                                nc.vector.tensor_copy(dst, tp[:])
                            else:
                                nc.scalar.copy(dst, tp[:])

                    for r in range(RD):
                        st6 = stp.tile([128, 12], f32, tag="st6",
                                       name=f"st6_{s}_{r}")
                        for half in range(NH):
                            sl = slice(half * 512, (half + 1) * 512)
                            p2 = ps2p.tile([128, 512], f32, tag="ps2",
                                           name=f"p2_{s}_{r}_{half}")
                            # xeg seeded into PSUM via identity matmul (PE is
                            # cheaper here than an extra DVE pass: the DVE is
                            # the step-boundary critical path)
                            nc.tensor.matmul(
                                p2[:], ident[:], xeg[r][:, sl],
                                start=True, stop=False)
                            for j in range(FP):
                                nc.tensor.matmul(
                                    p2[:],
                                    hidT[j][:, :, r * 128:(r + 1) * 128],
                                    w2[j][:, :, sl],
                                    start=False, stop=(j == FP - 1),
                                    perf_mode=DR)
                            nc.vector.scalar_tensor_tensor(
                                h[r][:, sl], h[r][:, sl], 1.0 - GAMMA, p2[:],
                                op0=OP.mult, op1=OP.add)
                            if not last:
                                nc.vector.bn_stats(
                                    st6[:, half * 6:(half + 1) * 6],
                                    h[r][:, sl])
                            if half == 0 and r >= 2:
                                # interleaved transposes of row r-2 (normalize
                                # long since drained; PE slots them between
                                # mm2 groups, casts go behind this row's stats)
                                transp_any(r - 2, f"{s}_{r - 2}")
                        if last:
                            # head prep inline: cast final h to fp16 so its
                            # transposes overlap the remaining matmul2 groups
                            hc16 = wk.tile([128, DH], f16, tag=f"hn16_{r}",
                                           name=f"hc16_{r}", bufs=1)
                            nc.vector.tensor_copy(hc16[:], h[r][:])
                            hns[r] = hc16
                            continue
                        nc.vector.bn_aggr(mvv[:, r], st6[:])
                        if fused:
                            a = stp.tile([128, 1], f32, tag="nwa",
                                         name=f"nfa_{s}_{r}")
                            nc.vector.tensor_scalar(
                                a[:], mv[:, 2 * r + 1:2 * r + 2],
                                nscv[:, 0, r:r + 1], nscv[:, 1, r:r + 1],
                                op0=OP.mult, op1=OP.add)
                            nc.vector.tensor_tensor(
                                rs[:, r:r + 1], rs_p[:, r:r + 1], a[:],
                                op=OP.mult)
                        else:
                            rstd_newton(rs[:, r:r + 1], rs_p[:, r:r + 1],
                                        mv[:, 2 * r + 1:2 * r + 2],
                                        f"{s}_{r}")
                        hns[r] = normalize(r, mv[:, 2 * r:2 * r + 1],
                                           rs[:, r:r + 1], f"{s}_{r}")
                    for r in (RD - 2, RD - 1):
                        transp_any(r, f"{s}_{r}")
                    rs_p = rs

                # ---- head: outT = (head_w/SH) @ h.T, out = po/SH' + hb ----
                for ot in range(OD):
                    po = ps1p.tile([128, 512], f32, tag="ps1", name=f"po_{ot}")
                    for k in range(KD):
                        nc.tensor.matmul(
                            po[:], hwt[k][:, ot * 128:(ot + 1) * 128],
                            hfT[k // 2][:, k % 2, :],
                            start=(k == 0), stop=(k == KD - 1))
                    osb = wk.tile([128, 512], f32, tag="osb",
                                  name=f"osb_{ot}", bufs=1)
                    nc.scalar.activation(
                        osb[:], po[:], AF.Identity, bias=hbs[:, ot:ot + 1],
                        scale=SH / S2)
                    nc.sync.dma_start(
                        outT_d.ap()[ot * 128:(ot + 1) * 128, :], osb[:])

    nc.compile()
    return nc


def _get_compiled(steps: int):
    key = ("prog", steps)
    if key not in _CACHE:
        from concourse.bass_interp import get_hw_module
        nc = _build_program(steps)
        nc.m = get_hw_module(nc.m)
        _CACHE[key] = nc
    return _CACHE[key]


def _spectral_sigma(W: np.ndarray) -> float:
    W = W.astype(np.float64)
    v = np.full(W.shape[1], 1.0 / np.sqrt(W.shape[1]))
    u = W @ v
    u = u / (np.linalg.norm(u) + 1e-12)
    for _ in range(15):
        u = W @ v
        u = u / (np.linalg.norm(u) + 1e-12)
        v = W.T @ u
        v = v / (np.linalg.norm(v) + 1e-12)
    return float(u @ (W @ v))


def _q8(x: np.ndarray) -> np.ndarray:
    import ml_dtypes
    return np.ascontiguousarray(
        np.clip(x, -240.0, 240.0).astype(ml_dtypes.float8_e4m3))


def _prep_host(inputs: dict) -> tuple[dict, list]:
    f = {k: np.asarray(v, dtype=np.float32) for k, v in inputs.items()}
    x, ew, eb = f["x"], f["embed_w"], f["embed_b"]
    W1, b1, W2, b2 = f["W1"], f["b1"], f["W2"], f["b2"]
    ln_g, ln_b = f["ln_g"], f["ln_b"]
    hw_, hb = f["head_w"], f["head_b"]

    s1 = _spectral_sigma(W1)
    s2 = _spectral_sigma(W2)
    W1n = (W1.astype(np.float64) / s1)
    W2n = (W2.astype(np.float64) / s2)
    # fold ln gain into W1, ln bias into b1
    W1eff = W1n * ln_g.astype(np.float64)[None, :]
    b1eff = (b1.astype(np.float64) + W1n @ ln_b.astype(np.float64))
    W2eff = GAMMA * W2n

    shared = {
        "ewT": np.ascontiguousarray(S2 * ew.T).astype(np.float16),
        "w1t": _q8(np.ascontiguousarray(S1 * W1eff.T)),
        "b1e": b1eff.astype(np.float32).reshape(DFF, 1),
        "w2t": _q8(np.ascontiguousarray(S2 * W2eff.T)),
        "vb1": (S2 * eb).reshape(1, DH).astype(np.float16),
        "vb2": (S2 * GAMMA * b2).reshape(1, DH).astype(np.float32),
        "hwt": np.ascontiguousarray(hw_.T / SH).astype(np.float16),
        "hb": hb.reshape(DOUT, 1).astype(np.float32),
    }
    in_maps = []
    for c in range(N_CORES):
        shard = x[c * R:(c + 1) * R, :]
        m = dict(shared)
        m["xT"] = np.ascontiguousarray(shard.T).astype(np.float16)
        in_maps.append(m)
    return shared, in_maps


def kernel(**inputs) -> np.ndarray:
    from concourse import bass_utils
    nc = _get_compiled(STEPS)
    _, in_maps = _prep_host(inputs)
    res = None
    for attempt in range(3):
        try:
            res = bass_utils.run_bass_kernel_spmd(
                nc, in_maps, core_ids=list(range(N_CORES)))
            break
        except Exception:
            # transient NRT_EXEC_UNIT_UNRECOVERABLE device wedges clear on
            # retry
            if attempt == 2:
                raise
    out = np.empty((B, DOUT), np.float32)
    for c in range(N_CORES):
        out[c * R:(c + 1) * R, :] = res.results[c]["outT"].T
    return out


if __name__ == "__main__":
    rng = np.random.default_rng(0)
    demo = {
        "x": rng.standard_normal((B, DIN)).astype(np.float32),
        "embed_w": (rng.standard_normal((DH, DIN)) * 0.02).astype(np.float32),
        "embed_b": np.zeros(DH, np.float32),
        "W1": (rng.standard_normal((DFF, DH)) * 0.02).astype(np.float32),
        "b1": np.zeros(DFF, np.float32),
        "W2": (rng.standard_normal((DH, DFF)) * 0.02).astype(np.float32),
        "b2": np.zeros(DH, np.float32),
        "ln_g": np.ones(DH, np.float32),
        "ln_b": np.zeros(DH, np.float32),
        "head_w": (rng.standard_normal((DOUT, DH)) * 0.02).astype(np.float32),
        "head_b": np.zeros(DOUT, np.float32),
    }
    out = kernel(**demo)
    print("out", out.shape, out.dtype, float(np.abs(out).max()))


# revision 18
# speedup vs baseline: 2.1482x; 1.0148x over previous
"""Bass/Tile TRN2 kernel for nn_DimensionScaledEqProp.

Data-parallel over batch: x rows sharded across 8 NeuronCores, weights
replicated. Per-core state (h) stays resident in SBUF across the 30
sequential steps.

fp8(e4m3) DoubleRow matmuls for the two big GEMMs (2 MACs/cell/cycle),
fp32 state/accumulation. h is carried scaled by S2=256 so the fp8 weight
scaling folds away: LN is scale-invariant (eps scaled to match), and the
head weights absorb 1/S2.

Self-contained: hardcodes shapes; host side does sharding, spectral-norm
sigma (tiny: 60 matvecs), weight folding/quantization, and the final
gather/transpose.
"""
import sys
import numpy as np

for _p in ("/opt/trn_rl_repo", "/root/.axon_site/_ro/trn_rl_repo"):
    if _p not in sys.path:
        sys.path.append(_p)

B, DIN, DH, DOUT = 4096, 512, 1024, 256
DFF = 4 * DH
STEPS = 30
N_CORES = 8
R = B // N_CORES  # rows per core = 512
GAMMA = 0.5 * min(1.0, float(np.sqrt(64.0 / DIN)))
LN_EPS = 1e-5

KD = DH // 128    # 8  k-tiles over DH
FD = DFF // 128   # 32 f-tiles over DFF
RD = R // 128     # 4  row-tiles per core
ID = DIN // 128   # 4  k-tiles over DIN
OD = DOUT // 128  # 2  out-tiles over DOUT
NH = DH // 512    # 2  psum halves over DH

S1 = 64.0         # fp8 scale on W1eff (undone by tanh's activation scale)
S2 = 256.0        # fp8 scale on W2eff, carried inside h
SH = 16.0         # partial head-weight scale (rest via activation scale)
EPS_DEV = LN_EPS * S2 * S2

KP = KD // 2      # 4  DoubleRow k-pairs for mm1
FP = FD // 2      # 16 DoubleRow f-pairs for mm2

_CACHE = {}


def _build_program(steps: int):
    import concourse.bass as bass
    import concourse.bacc as bacc
    import concourse.mybir as mybir
    from concourse import tile, masks

    f8 = mybir.dt.float8e4
    f16 = mybir.dt.float16
    f32 = mybir.dt.float32
    AF = mybir.ActivationFunctionType
    OP = mybir.AluOpType
    DR = mybir.MatmulPerfMode.DoubleRow

    nc = bacc.Bacc("TRN2", target_bir_lowering=False, debug=False,
                   enable_asserts=True, num_devices=N_CORES)

    xT_d = nc.dram_tensor("xT", [DIN, R], f16, kind="ExternalInput")
    ewT_d = nc.dram_tensor("ewT", [DIN, DH], f16, kind="ExternalInput")
    w1t_d = nc.dram_tensor("w1t", [DH, DFF], f8, kind="ExternalInput")
    b1e_d = nc.dram_tensor("b1e", [DFF, 1], f32, kind="ExternalInput")
    w2t_d = nc.dram_tensor("w2t", [DFF, DH], f8, kind="ExternalInput")
    vb1_d = nc.dram_tensor("vb1", [1, DH], f16, kind="ExternalInput")
    vb2_d = nc.dram_tensor("vb2", [1, DH], f32, kind="ExternalInput")
    hwt_d = nc.dram_tensor("hwt", [DH, DOUT], f16, kind="ExternalInput")
    hb_d = nc.dram_tensor("hb", [DOUT, 1], f32, kind="ExternalInput")
    outT_d = nc.dram_tensor("outT", [DOUT, R], f32, kind="ExternalOutput")

    with tile.TileContext(nc) as tc:
        with (
            tc.tile_pool(name="wp", bufs=1) as wp,
            tc.tile_pool(name="sp", bufs=1) as sp,
            tc.tile_pool(name="wk", bufs=2) as wk,
            tc.tile_pool(name="stp", bufs=2) as stp,
            tc.tile_pool(name="pst", bufs=4, space="PSUM") as pst,
            tc.tile_pool(name="ps1", bufs=2, space="PSUM") as ps1p,
            tc.tile_pool(name="ps2", bufs=2, space="PSUM") as ps2p,
        ):
            # ---- persistent weights / constants ----
            # DoubleRow pair layout: [128, 2, free]; dim1 = second k-subtile
            w1 = [wp.tile([128, 2, DFF], f8, name=f"w1_{j}") for j in range(KP)]
            w2 = [wp.tile([128, 2, DH], f8, name=f"w2_{j}") for j in range(FP)]
            hwt = [wp.tile([128, DOUT], f16, name=f"hwt_{k}") for k in range(KD)]
            b1s = wp.tile([128, FD], f32, name="b1s")
            hbs = wp.tile([128, OD], f32, name="hbs")
            ident = wp.tile([128, 128], f16, name="ident")

            # ---- persistent state ----
            h = [sp.tile([128, DH], f32, name=f"h_{r}") for r in range(RD)]
            xeg = [sp.tile([128, DH], f16, name=f"xeg_{r}") for r in range(RD)]
            hnT = [sp.tile([128, 2, R], f8, name=f"hnT_{j}") for j in range(KP)]
            hfT = [sp.tile([128, 2, R], f16, name=f"hfT_{j}")
                   for j in range(KP)]

            masks.make_identity(nc, ident[:])

            # ---- embed (transient pool, released before the step loop) ----
            mv0 = stp.tile([128, RD * 2], f32, tag="mv", name="mv_init")
            with tc.tile_pool(name="ep", bufs=1) as ep:
                xts = [ep.tile([128, R], f16, name=f"xts_{i}")
                       for i in range(ID)]
                ewt = [ep.tile([128, DH], f16, name=f"ewt_{i}")
                       for i in range(ID)]
                bc1 = ep.tile([128, DH], f16, name="bc1")
                bc2 = ep.tile([128, DH], f32, name="bc2")
                for i in range(ID):
                    nc.sync.dma_start(
                        xts[i][:], xT_d.ap()[i * 128:(i + 1) * 128, :])
                    nc.sync.dma_start(
                        ewt[i][:], ewT_d.ap()[i * 128:(i + 1) * 128, :])
                nc.sync.dma_start(bc1[0:1, :], vb1_d.ap())
                nc.sync.dma_start(bc2[0:1, :], vb2_d.ap())
                nc.gpsimd.partition_broadcast(bc1[:], bc1[0:1, :])
                nc.gpsimd.partition_broadcast(bc2[:], bc2[0:1, :])

                # weight loads AFTER embed inputs: embed matmuls start
                # immediately; w1/w2 stream in behind them
                for j in range(KP):
                    for i2 in range(2):
                        k = 2 * j + i2
                        nc.sync.dma_start(
                            w1[j][:, i2, :],
                            w1t_d.ap()[k * 128:(k + 1) * 128, :])
                nc.sync.dma_start(
                    b1s[:], b1e_d.ap().rearrange("(f p) o -> p (f o)", p=128))
                nc.sync.dma_start(
                    hbs[:], hb_d.ap().rearrange("(t p) o -> p (t o)", p=128))

                # h0 = x @ ewT + embed_b (bias seeded into PSUM on the PE,
                # PSUM->SBUF copy on ACT, stats on DVE: three-engine pipeline)
                for r in range(RD):
                    for half in range(NH):
                        sl = slice(half * 512, (half + 1) * 512)
                        pe = ps1p.tile([128, 512], f32, tag="ps1", name="pe")
                        nc.tensor.matmul(
                            pe[:], ident[:], bc1[:, sl],
                            start=True, stop=False)
                        for i in range(ID):
                            nc.tensor.matmul(
                                pe[:], xts[i][:, r * 128:(r + 1) * 128],
                                ewt[i][:, sl],
                                start=False, stop=(i == ID - 1))
                        nc.scalar.copy(h[r][:, sl], pe[:])
                    st6 = stp.tile([128, 12], f32, tag="st6",
                                   name=f"st6_i_{r}")
                    for c in range(2):
                        nc.vector.bn_stats(
                            st6[:, c * 6:(c + 1) * 6],
                            h[r][:, c * 512:(c + 1) * 512])
                    nc.vector.bn_aggr(
                        mv0[:].rearrange("p (r x) -> p r x", x=2)[:, r],
                        st6[:])
                # xeg = g*h0 + g*S2*b2 (f16) — off the init-LN critical path
                for r in range(RD):
                    for half in range(NH):
                        sl = slice(half * 512, (half + 1) * 512)
                        nc.vector.scalar_tensor_tensor(
                            xeg[r][:, sl], h[r][:, sl], GAMMA, bc2[:, sl],
                            op0=OP.mult, op1=OP.add)

            # ---- initial rstd (ACT sqrt once; hides table load) ----
            ve0 = stp.tile([128, RD], f32, tag="ve", name="ve_init")
            nc.vector.tensor_scalar(
                ve0[:], mv0[:].rearrange("p (r x) -> p x r", x=2)[:, 1],
                EPS_DEV, None, op0=OP.add)
            rv0 = stp.tile([128, RD], f32, tag="rv", name="rv_init")
            nc.vector.reciprocal(rv0[:], ve0[:])
            rs_prev = stp.tile([128, RD], f32, tag="rs", name="rs_init")
            nc.scalar.activation(rs_prev[:], rv0[:], AF.Sqrt)

            # ---- hidT pool reuses the embed pool space ----
            with tc.tile_pool(name="hp", bufs=1) as hp:
                hidT = [hp.tile([128, 2, R], f8, name=f"hidT_{j}")
                        for j in range(FP)]

                def rstd_newton(y_out, y_seed, var_ap, tag_sfx, n_iter=2):
                    """y_out[128,1] = 1/sqrt(var+eps) via Newton from seed."""
                    hv = stp.tile([128, 1], f32, tag="hv",
                                  name=f"hv_{tag_sfx}")
                    nc.vector.tensor_scalar(
                        hv[:], var_ap, -0.5, -0.5 * EPS_DEV,
                        op0=OP.mult, op1=OP.add)
                    y = y_seed
                    for it in range(n_iter):
                        a = stp.tile([128, 1], f32, tag="nwa",
                                     name=f"nwa_{tag_sfx}_{it}")
                        nc.vector.tensor_tensor(a[:], y, y, op=OP.mult)
                        nc.vector.tensor_scalar(
                            a[:], a[:], hv[:], 1.5, op0=OP.mult, op1=OP.add)
                        if it == n_iter - 1:
                            nc.vector.tensor_tensor(y_out, y, a[:], op=OP.mult)
                        else:
                            yn = stp.tile([128, 1], f32, tag="nwy",
                                          name=f"nwy_{tag_sfx}_{it}")
                            nc.vector.tensor_tensor(yn[:], y, a[:], op=OP.mult)
                            y = yn[:]

                # normalize h0 -> hnT for step 0 (one Newton polish on seed)
                rs_fix = stp.tile([128, RD], f32, tag="rsf", name="rs_fix")
                for r in range(RD):
                    rstd_newton(rs_fix[:, r:r + 1], rs_prev[:, r:r + 1],
                                mv0[:, 2 * r + 1:2 * r + 2], f"i{r}", n_iter=1)
                rs_p = rs_fix

                def normalize(r, mean_ap, rs_col, sfx):
                    nmu = stp.tile([128, 1], f32, tag="nmu", name=f"nmu_{sfx}")
                    nc.vector.scalar_tensor_tensor(
                        nmu[:], mean_ap, -1.0, rs_col,
                        op0=OP.mult, op1=OP.mult)
                    hn16 = wk.tile([128, DH], f16, tag=f"hn16_{r}",
                                   name=f"hn16_{sfx}", bufs=1)
                    # two half-width ops so transposes of the low half can
                    # start before the high half is normalized
                    for half in range(NH):
                        sl = slice(half * 512, (half + 1) * 512)
                        nc.vector.tensor_scalar(
                            hn16[:, sl], h[r][:, sl], rs_col, nmu[:],
                            op0=OP.mult, op1=OP.add)
                    return hn16

                def transposes(r, hn16, sfx):
                    # paired f16 PE transposes into one [128,256] PSUM tile;
                    # single PSUM->SBUF cast writes the fp8 DoubleRow pair.
                    # rows 2/3 cast on ACT only: their casts land at the step
                    # boundary where the DVE queue is the critical path
                    for j in range(KP):
                        tp = pst.tile([128, 256], f16, tag="tp",
                                      name=f"tp_{sfx}_{j}")
                        for i2 in range(2):
                            k = 2 * j + i2
                            nc.tensor.transpose(
                                tp[:, i2 * 128:(i2 + 1) * 128],
                                hn16[:, k * 128:(k + 1) * 128], ident[:])
                        dst = hnT[j][:, :, r * 128:(r + 1) * 128]
                        if r == 2 or (r == 3 and j % 2 == 1) or \
                                (r < 2 and j % 2 == 1):
                            nc.scalar.copy(dst, tp[:])
                        else:
                            nc.vector.tensor_copy(dst, tp[:])

                for r in range(RD):
                    hn = normalize(r, mv0[:, 2 * r:2 * r + 1],
                                   rs_p[:, r:r + 1], f"s0_{r}")
                    transposes(r, hn, f"s0_{r}")

                for s in range(steps):
                    last = (s == steps - 1)
                    # mm1: hidT = tanh((W1s' @ hnT)/S1 + b1)  [DoubleRow fp8]
                    for f in range(FD):
                        p1 = ps1p.tile([128, 512], f32, tag="ps1",
                                       name=f"p1_{s}_{f}")
                        for j in range(KP):
                            nc.tensor.matmul(
                                p1[:], w1[j][:, :, f * 128:(f + 1) * 128],
                                hnT[j][:],
                                start=(j == 0), stop=(j == KP - 1),
                                perf_mode=DR)
                        nc.scalar.activation(
                            hidT[f // 2][:, f % 2, :], p1[:], AF.Tanh,
                            bias=b1s[:, f:f + 1], scale=1.0 / S1)

                    if s == 0:
                        # w2/hwt loads deferred past step-0 mm1 so w1 gets
                        # full DMA bandwidth at startup; w2 arrives during
                        # mm1 execution, well before mm2 needs it
                        for j in range(FP):
                            for i2 in range(2):
                                f_ = 2 * j + i2
                                nc.sync.dma_start(
                                    w2[j][:, i2, :],
                                    w2t_d.ap()[f_ * 128:(f_ + 1) * 128, :])
                        for k_ in range(KD):
                            nc.sync.dma_start(
                                hwt[k_][:],
                                hwt_d.ap()[k_ * 128:(k_ + 1) * 128, :])

                    # mm2 per (row-tile, half): p2 = W2s' @ hidT [DoubleRow],
                    # then h += p2; stats/rstd/normalize/transpose interleaved.
                    # transp(r0)/transp(r1) slot between later mm2 groups
                    # (their normalizes have drained by then); r2+r3 stay at
                    # the step end as PE cover while r3's DVE chain drains.
                    mv = stp.tile([128, RD * 2], f32, tag="mv",
                                  name=f"mv_{s}")
                    mvv = mv[:].rearrange("p (r x) -> p r x", x=2)
                    rs = stp.tile([128, RD], f32, tag="rs", name=f"rs_{s}")
                    hns = {}

                    # fused 1-iter Newton for rstd once var drift is small
                    # (s>=4): rs = y*(var*s1 + s2), s1/s2 precomputed from the
                    # seed y during mm1 so only 2 small DVE ops sit on the
                    # step-boundary critical path.
                    fused = (s >= 1) and not last
                    if fused:
                        nsc = stp.tile([128, 2 * RD], f32, tag="nsc",
                                       name=f"nsc_{s}")
                        yy = stp.tile([128, RD], f32, tag="nyy",
                                      name=f"nyy_{s}")
                        nc.vector.tensor_tensor(
                            yy[:], rs_p[:], rs_p[:], op=OP.mult)
                        nscv = nsc[:].rearrange("p (x r) -> p x r", x=2)
                        nc.vector.tensor_scalar(
                            nscv[:, 0], yy[:], -0.5, None, op0=OP.mult)
                        nc.vector.tensor_scalar(
                            nscv[:, 1], yy[:], -0.5 * EPS_DEV, 1.5,
                            op0=OP.mult, op1=OP.add)

                    def transp_any(r, sfx):
                        if last:
                            transposes_head(r, hns[r], sfx)
                        else:
                            transposes(r, hns[r], sfx)

                    def transposes_head(r, hn16, sfx):
                        for j in range(KP):
                            tp = pst.tile([128, 256], f16, tag="tp",
                                          name=f"tpf_{sfx}_{j}")
                            for i2 in range(2):
                                k = 2 * j + i2
                                nc.tensor.transpose(
                                    tp[:, i2 * 128:(i2 + 1) * 128],
                                    hn16[:, k * 128:(k + 1) * 128], ident[:])
                            dst = hfT[j][:, :, r * 128:(r + 1) * 128]
                            if j % 2 == 0:
                                nc.vector.tensor_copy(dst, tp[:])
                            else:
                                nc.scalar.copy(dst, tp[:])

                    for r in range(RD):
                        st6 = stp.tile([128, 12], f32, tag="st6",
                                       name=f"st6_{s}_{r}")
                        for half in range(NH):
                            sl = slice(half * 512, (half + 1) * 512)
                            p2 = ps2p.tile([128, 512], f32, tag="ps2",
                                           name=f"p2_{s}_{r}_{half}")
                            # xeg seeded into PSUM via identity matmul (PE is
                            # cheaper here than an extra DVE pass: the DVE is
                            # the step-boundary critical path)
                            nc.tensor.matmul(
                                p2[:], ident[:], xeg[r][:, sl],
                                start=True, stop=False)
                            for j in range(FP):
                                nc.tensor.matmul(
                                    p2[:],
                                    hidT[j][:, :, r * 128:(r + 1) * 128],
                                    w2[j][:, :, sl],
                                    start=False, stop=(j == FP - 1),
                                    perf_mode=DR)
                            nc.vector.scalar_tensor_tensor(
                                h[r][:, sl], h[r][:, sl], 1.0 - GAMMA, p2[:],
                                op0=OP.mult, op1=OP.add)
                            if not last:
                                nc.vector.bn_stats(
                                    st6[:, half * 6:(half + 1) * 6],
                                    h[r][:, sl])
                            if half == 0 and r >= 2:
                                # interleaved transposes of row r-2 (normalize
                                # long since drained; PE slots them between
                                # mm2 groups, casts go behind this row's stats)
                                transp_any(r - 2, f"{s}_{r - 2}")
                        if last:
                            # head prep inline: cast final h to fp16 so its
                            # transposes overlap the remaining matmul2 groups
                            hc16 = wk.tile([128, DH], f16, tag=f"hn16_{r}",
                                           name=f"hc16_{r}", bufs=1)
                            nc.vector.tensor_copy(hc16[:], h[r][:])
                            hns[r] = hc16
                            continue
                        nc.vector.bn_aggr(mvv[:, r], st6[:])
                        if fused:
                            a = stp.tile([128, 1], f32, tag="nwa",
                                         name=f"nfa_{s}_{r}")
                            nc.vector.tensor_scalar(
                                a[:], mv[:, 2 * r + 1:2 * r + 2],
                                nscv[:, 0, r:r + 1], nscv[:, 1, r:r + 1],
                                op0=OP.mult, op1=OP.add)
                            nc.vector.tensor_tensor(
                                rs[:, r:r + 1], rs_p[:, r:r + 1], a[:],
                                op=OP.mult)
                        else:
                            rstd_newton(rs[:, r:r + 1], rs_p[:, r:r + 1],
                                        mv[:, 2 * r + 1:2 * r + 2],
                                        f"{s}_{r}")
                        hns[r] = normalize(r, mv[:, 2 * r:2 * r + 1],
                                           rs[:, r:r + 1], f"{s}_{r}")
                    for r in (RD - 2, RD - 1):
                        transp_any(r, f"{s}_{r}")
                    rs_p = rs

                # ---- head: outT = (head_w/SH) @ h.T, out = po/SH' + hb ----
                for ot in range(OD):
                    po = ps1p.tile([128, 512], f32, tag="ps1", name=f"po_{ot}")
                    for k in range(KD):
                        nc.tensor.matmul(
                            po[:], hwt[k][:, ot * 128:(ot + 1) * 128],
                            hfT[k // 2][:, k % 2, :],
                            start=(k == 0), stop=(k == KD - 1))
                    osb = wk.tile([128, 512], f32, tag="osb",
                                  name=f"osb_{ot}", bufs=1)
                    nc.scalar.activation(
                        osb[:], po[:], AF.Identity, bias=hbs[:, ot:ot + 1],
                        scale=SH / S2)
                    nc.sync.dma_start(
                        outT_d.ap()[ot * 128:(ot + 1) * 128, :], osb[:])

    nc.compile()
    return nc


def _get_compiled(steps: int):
    key = ("prog", steps)
    if key not in _CACHE:
        from concourse.bass_interp import get_hw_module
        nc = _build_program(steps)
        nc.m = get_hw_module(nc.m)
        _CACHE[key] = nc
    return _CACHE[key]


def _spectral_sigma(W: np.ndarray) -> float:
    W = W.astype(np.float64)
    v = np.full(W.shape[1], 1.0 / np.sqrt(W.shape[1]))
    u = W @ v
    u = u / (np.linalg.norm(u) + 1e-12)
    for _ in range(15):
        u = W @ v
        u = u / (np.linalg.norm(u) + 1e-12)
        v = W.T @ u
        v = v / (np.linalg.norm(v) + 1e-12)
    return float(u @ (W @ v))


def _q8(x: np.ndarray) -> np.ndarray:
    import ml_dtypes
    return np.ascontiguousarray(
        np.clip(x, -240.0, 240.0).astype(ml_dtypes.float8_e4m3))


def _prep_host(inputs: dict) -> tuple[dict, list]:
    f = {k: np.asarray(v, dtype=np.float32) for k, v in inputs.items()}
    x, ew, eb = f["x"], f["embed_w"], f["embed_b"]
    W1, b1, W2, b2 = f["W1"], f["b1"], f["W2"], f["b2"]
    ln_g, ln_b = f["ln_g"], f["ln_b"]
    hw_, hb = f["head_w"], f["head_b"]

    s1 = _spectral_sigma(W1)
    s2 = _spectral_sigma(W2)
    W1n = (W1.astype(np.float64) / s1)
    W2n = (W2.astype(np.float64) / s2)
    # fold ln gain into W1, ln bias into b1
    W1eff = W1n * ln_g.astype(np.float64)[None, :]
    b1eff = (b1.astype(np.float64) + W1n @ ln_b.astype(np.float64))
    W2eff = GAMMA * W2n

    shared = {
        "ewT": np.ascontiguousarray(S2 * ew.T).astype(np.float16),
        "w1t": _q8(np.ascontiguousarray(S1 * W1eff.T)),
        "b1e": b1eff.astype(np.float32).reshape(DFF, 1),
        "w2t": _q8(np.ascontiguousarray(S2 * W2eff.T)),
        "vb1": (S2 * eb).reshape(1, DH).astype(np.float16),
        "vb2": (S2 * GAMMA * b2).reshape(1, DH).astype(np.float32),
        "hwt": np.ascontiguousarray(hw_.T / SH).astype(np.float16),
        "hb": hb.reshape(DOUT, 1).astype(np.float32),
    }
    in_maps = []
    for c in range(N_CORES):
        shard = x[c * R:(c + 1) * R, :]
        m = dict(shared)
        m["xT"] = np.ascontiguousarray(shard.T).astype(np.float16)
        in_maps.append(m)
    return shared, in_maps


def kernel(**inputs) -> np.ndarray:
    from concourse import bass_utils
    nc = _get_compiled(STEPS)
    _, in_maps = _prep_host(inputs)
    res = None
    for attempt in range(3):
        try:
            res = bass_utils.run_bass_kernel_spmd(
                nc, in_maps, core_ids=list(range(N_CORES)))
            break
        except Exception:
            # transient NRT_EXEC_UNIT_UNRECOVERABLE device wedges clear on
            # retry
            if attempt == 2:
                raise
    out = np.empty((B, DOUT), np.float32)
    for c in range(N_CORES):
        out[c * R:(c + 1) * R, :] = res.results[c]["outT"].T
    return out


if __name__ == "__main__":
    rng = np.random.default_rng(0)
    demo = {
        "x": rng.standard_normal((B, DIN)).astype(np.float32),
        "embed_w": (rng.standard_normal((DH, DIN)) * 0.02).astype(np.float32),
        "embed_b": np.zeros(DH, np.float32),
        "W1": (rng.standard_normal((DFF, DH)) * 0.02).astype(np.float32),
        "b1": np.zeros(DFF, np.float32),
        "W2": (rng.standard_normal((DH, DFF)) * 0.02).astype(np.float32),
        "b2": np.zeros(DH, np.float32),
        "ln_g": np.ones(DH, np.float32),
        "ln_b": np.zeros(DH, np.float32),
        "head_w": (rng.standard_normal((DOUT, DH)) * 0.02).astype(np.float32),
        "head_b": np.zeros(DOUT, np.float32),
    }
    out = kernel(**demo)
    print("out", out.shape, out.dtype, float(np.abs(out).max()))
